# revision 1
# baseline (speedup 1.0000x reference)
"""Trainium2 Bass kernel for an 8-block linear-attention transformer.

Contract: kernel(**inputs) takes full unsharded inputs (as in
reference.setup_inputs()) and returns the full [N, S, D] output.

Sharding: sequence-parallel over the 16384 tokens -> 2048 tokens/core on
8 NeuronCores. The only cross-token coupling is the per-sample KV/sumK
sums of the linear attention; each core computes partial sums over its
local tokens and a tiny [128, 64] AllReduce per block produces the
global sums. Everything else is purely token-parallel.

On-device layout is feature-major ([feature partitions, tokens free]) so
no transposes are ever needed: matmuls take activations as the moving
operand and weights as the stationary operand, producing feature-major
outputs. All matmuls run in float32r (full-rate fp32, ~1.4e-4).
"""

import sys

sys.path.insert(0, "/opt/trn_rl_repo")

import numpy as np

# dims (hardcoded; must match reference.py)
B, H, D, K, F = 8, 16, 1024, 64, 4096
N, S = 4, 4096
HK = H * K  # 1024
N_CORES = 8
TPC = (N * S) // N_CORES  # tokens per core = 2048
SEQ_SH = S // N_CORES     # 512 local tokens per sample
NC_CHUNK = TPC // N       # 512 = one sample's local tokens (= chunk)
ND = D // 128             # 8 d-tiles
NO = HK // 128            # 8 attention-feature tiles
NF = F // 128             # 32 ffn tiles
LN_EPS = 1e-6
ATT_EPS = 1e-6

# bias/scale column layout in the packed per-block scalar tensor
_COLS = {}
_c = 0
for _name, _n in [("bq", NO), ("bk", NO), ("bv", NO), ("bo", ND),
                  ("ln1_s", ND), ("ln1_b", ND), ("ln2_s", ND), ("ln2_b", ND),
                  ("b2", ND), ("b1", NF)]:
    _COLS[_name] = _c
    _c += _n
NSCAL = _c  # 104

_BUILD_CACHE = {}


def _build(n_blocks, debug=False):
    """Build the SPMD bass program (same program for all 8 cores)."""
    import concourse.bacc as bacc
    import concourse.tile as tile
    import concourse.mybir as mybir

    F32 = mybir.dt.float32
    F32R = mybir.dt.float32r
    AF = mybir.ActivationFunctionType
    ALU = mybir.AluOpType
    AX = mybir.AxisListType

    nc = bacc.Bacc("TRN2", target_bir_lowering=False, debug=False,
                   num_devices=N_CORES)

    x_ap = nc.dram_tensor("x_fm", [D, TPC], F32, kind="ExternalInput").ap()
    wq_ap = nc.dram_tensor("wq", [n_blocks, D, HK], F32, kind="ExternalInput").ap()
    wk_ap = nc.dram_tensor("wk", [n_blocks, D, HK], F32, kind="ExternalInput").ap()
    wv_ap = nc.dram_tensor("wv", [n_blocks, D, HK], F32, kind="ExternalInput").ap()
    wo_ap = nc.dram_tensor("wo", [n_blocks, HK, D], F32, kind="ExternalInput").ap()
    w1_ap = nc.dram_tensor("w1", [n_blocks, D, F], F32, kind="ExternalInput").ap()
    w2_ap = nc.dram_tensor("w2", [n_blocks, F, D], F32, kind="ExternalInput").ap()
    sc_ap = nc.dram_tensor("scal", [n_blocks, 128, NSCAL], F32, kind="ExternalInput").ap()
    out_ap = nc.dram_tensor("out_fm", [D, TPC], F32, kind="ExternalOutput").ap()
    dbg_aps = {}
    if debug:
        for nm, shp in [("d_kvsloc", [128, 64]), ("d_kvs", [128, 64]),
                        ("d_att0", [128, TPC]), ("d_z1", [128, TPC]),
                        ("d_ln1", [128, TPC]), ("d_ffnz", [128, TPC]),
                        ("d_h0", [128, 512]), ("d_zacc", [128, TPC])]:
            dbg_aps[nm] = nc.dram_tensor(nm, shp, F32, kind="ExternalOutput").ap()

    def col_dma(dst_tile, src2d, ktiles):
        """DMA a [ktiles*128, 128] DRAM slice into a [128, ktiles*128]
        SBUF tile (k-tiles side by side), casting f32 -> f32r."""
        nc.gpsimd.dma_start(
            dst_tile[:].rearrange("p (a o) -> p a o", o=128),
            src2d.rearrange("(a p) o -> p a o", p=128),
        )

    with tile.TileContext(nc) as tc:
        with (
            tc.tile_pool(name="resid", bufs=1) as resid,
            tc.tile_pool(name="consts", bufs=1) as consts,
            tc.tile_pool(name="scal", bufs=2) as scalp,
            tc.tile_pool(name="dram", bufs=2, space="DRAM") as dramp,
        ):
            # persistent residual-stream buffers (feature-major)
            X = [resid.tile([128, TPC], F32R, name=f"X{d}", tag=f"X{d}")
                 for d in range(ND)]
            Z = [resid.tile([128, TPC], F32R, name=f"Z{d}", tag=f"Z{d}")
                 for d in range(ND)]

            # constants
            ones_col_f = consts.tile([128, 1], F32)
            nc.vector.memset(ones_col_f[:], 1.0 / D)
            ones_col = consts.tile([128, 1], F32R)   # stats lhsT (1/D scaling)
            nc.scalar.activation(ones_col[:], ones_col_f[:], AF.Copy)
            ones_row_f = consts.tile([1, 128], F32)
            nc.vector.memset(ones_row_f[:], 1.0)
            ones_row = consts.tile([1, 128], F32R)   # broadcast lhsT
            nc.scalar.activation(ones_row[:], ones_row_f[:], AF.Copy)
            eps_col = consts.tile([128, 1], F32)
            nc.vector.memset(eps_col[:], LN_EPS)

            # load input activations (cast to f32r)
            for d in range(ND):
                nc.gpsimd.dma_start(X[d][:], x_ap[d * 128:(d + 1) * 128, :])

            for b in range(n_blocks):
                sc_sb = scalp.tile([128, NSCAL], F32, name=f"sc{b}", tag="sc")
                nc.sync.dma_start(sc_sb[:], sc_ap[b])

                def scol(name, i):
                    return sc_sb[:, _COLS[name] + i:_COLS[name] + i + 1]

                # ---------------- Phase A: K, V -> local KV/sumK ----------
                kvs_loc = scalp.tile([128, 2 * NO * N], F32, name=f"kvl{b}", tag="kvl")
                # col layout: kv[o][n] at o*8+n, sumk[o][n] at o*8+4+n
                with (
                    tc.tile_pool(name="wkv", bufs=2) as wkvp,
                    tc.tile_pool(name="ascr", bufs=2) as ascr,
                    tc.tile_pool(name="apsum", bufs=2, space="PSUM") as apsum,
                ):
                    for o in range(NO):
                        wk_sb = wkvp.tile([128, D], F32R, name=f"wk{b}_{o}", tag="wk")
                        col_dma(wk_sb, wk_ap[b, :, o * 128:(o + 1) * 128], ND)
                        wv_sb = wkvp.tile([128, D], F32R, name=f"wv{b}_{o}", tag="wv")
                        col_dma(wv_sb, wv_ap[b, :, o * 128:(o + 1) * 128], ND)
                        for n in range(N):
                            t0, t1 = n * NC_CHUNK, (n + 1) * NC_CHUNK
                            ps_k = apsum.tile([128, NC_CHUNK], F32, name=f"psk{b}_{o}_{n}", tag="psk")
                            ps_v = apsum.tile([128, NC_CHUNK], F32, name=f"psv{b}_{o}_{n}", tag="psv")
                            for d in range(ND):
                                nc.tensor.matmul(
                                    ps_k[:], wk_sb[:, d * 128:(d + 1) * 128],
                                    X[d][:, t0:t1], start=(d == 0), stop=(d == ND - 1))
                            for d in range(ND):
                                nc.tensor.matmul(
                                    ps_v[:], wv_sb[:, d * 128:(d + 1) * 128],
                                    X[d][:, t0:t1], start=(d == 0), stop=(d == ND - 1))
                            # kf = exp(min(k+bk,0)) + relu(k+bk); sumk = sum(kf)
                            t1t = ascr.tile([128, NC_CHUNK], F32, name=f"t1_{b}_{o}_{n}", tag="t1")
                            nc.vector.tensor_scalar(t1t[:], ps_k[:], scol("bk", o), 0.0, ALU.add, ALU.min)
                            nc.scalar.activation(t1t[:], t1t[:], AF.Exp)
                            t2t = ascr.tile([128, NC_CHUNK], F32, name=f"t2_{b}_{o}_{n}", tag="t2")
                            nc.scalar.activation(t2t[:], ps_k[:], AF.Relu, bias=scol("bk", o))
                            kf = ascr.tile([128, NC_CHUNK], F32, name=f"kf_{b}_{o}_{n}", tag="kf")
                            nc.vector.scalar_tensor_tensor(
                                kf[:], t1t[:], 0.0, t2t[:], ALU.add, ALU.add,
                                accum_out=kvs_loc[:, o * 8 + 4 + n:o * 8 + 5 + n])
                            # kv = sum(kf * v_raw)
                            scr = ascr.tile([128, NC_CHUNK], F32, name=f"scr_{b}_{o}_{n}", tag="scr")
                            nc.vector.tensor_tensor(scr[:], kf[:], ps_v[:], ALU.mult)
                            nc.vector.tensor_reduce(
                                kvs_loc[:, o * 8 + n:o * 8 + 1 + n], scr[:],
                                axis=AX.X, op=ALU.add)

                # ---------------- AllReduce of KV/sumK --------------------
                cc_in = dramp.tile([128, 2 * NO * N], F32, name=f"cci{b}", tag="cci")
                cc_out = dramp.tile([128, 2 * NO * N], F32, name=f"cco{b}", tag="cco")
                nc.sync.dma_start(cc_in[:], kvs_loc[:])
                nc.gpsimd.collective_compute(
                    "AllReduce", mybir.AluOpType.add,
                    replica_groups=[list(range(N_CORES))],
                    ins=[cc_in.opt()], outs=[cc_out.opt()])
                kvs = scalp.tile([128, 2 * NO * N], F32, name=f"kvg{b}", tag="kvg")
                nc.sync.dma_start(kvs[:], cc_out[:])
                # fold V-bias into KV: kv += bv * sumk  (exact)
                for o in range(NO):
                    nc.vector.scalar_tensor_tensor(
                        kvs[:, o * 8:o * 8 + 4], kvs[:, o * 8 + 4:o * 8 + 8],
                        scol("bv", o), kvs[:, o * 8:o * 8 + 4], ALU.mult, ALU.add)

                if debug and b == 0:
                    nc.sync.dma_start(dbg_aps["d_kvsloc"][:], kvs_loc[:])
                    nc.sync.dma_start(dbg_aps["d_kvs"][:], kvs[:])

                # ------------- Phase B/C: Q -> att -> Wo -> z = y + x ------
                with (
                    tc.tile_pool(name="wqo", bufs=2) as wqop,
                    tc.tile_pool(name="bscr", bufs=2) as bscr,
                    tc.tile_pool(name="attp", bufs=2) as attp,
                    tc.tile_pool(name="bpsum", bufs=2, space="PSUM") as bpsum,
                ):
                    for n in range(N):
                        t0, t1 = n * NC_CHUNK, (n + 1) * NC_CHUNK
                        att = []
                        for o in range(NO):
                            wq_sb = wqop.tile([128, D], F32R, name=f"wq{b}_{o}_{n}", tag="wq")
                            col_dma(wq_sb, wq_ap[b, :, o * 128:(o + 1) * 128], ND)
                            ps_q = bpsum.tile([128, NC_CHUNK], F32, name=f"psq{b}_{o}_{n}", tag="psq")
                            for d in range(ND):
                                nc.tensor.matmul(
                                    ps_q[:], wq_sb[:, d * 128:(d + 1) * 128],
                                    X[d][:, t0:t1], start=(d == 0), stop=(d == ND - 1))
                            t1t = bscr.tile([128, NC_CHUNK], F32, name=f"bt1_{b}_{o}_{n}", tag="bt1")
                            nc.vector.tensor_scalar(t1t[:], ps_q[:], scol("bq", o), 0.0, ALU.add, ALU.min)
                            nc.scalar.activation(t1t[:], t1t[:], AF.Exp)
                            t2t = bscr.tile([128, NC_CHUNK], F32, name=f"bt2_{b}_{o}_{n}", tag="bt2")
                            nc.scalar.activation(t2t[:], ps_q[:], AF.Relu, bias=scol("bq", o))
                            qf = bscr.tile([128, NC_CHUNK], F32, name=f"qf_{b}_{o}_{n}", tag="qf")
                            nc.vector.scalar_tensor_tensor(
                                qf[:], t1t[:], 0.0, t2t[:], ALU.add, ALU.add)
                            den = bscr.tile([128, NC_CHUNK], F32, name=f"den_{b}_{o}_{n}", tag="den")
                            nc.vector.tensor_scalar(
                                den[:], qf[:], kvs[:, o * 8 + 4 + n:o * 8 + 5 + n],
                                ATT_EPS, ALU.mult, ALU.add)
                            rec = bscr.tile([128, NC_CHUNK], F32, name=f"rec_{b}_{o}_{n}", tag="rec")
                            nc.vector.reciprocal_approx_fast(rec[:], den[:])
                            at = attp.tile([128, NC_CHUNK], F32R, name=f"att{b}_{o}_{n}", tag=f"att{o}")
                            nc.vector.scalar_tensor_tensor(
                                at[:], qf[:], kvs[:, o * 8 + n:o * 8 + 1 + n],
                                rec[:], ALU.mult, ALU.mult)
                            if debug and b == 0 and o == 0:
                                nc.sync.dma_start(dbg_aps["d_att0"][:, t0:t1], at[:].bitcast(F32))
                            att.append(at)
                        for o2 in range(ND):
                            wo_sb = wqop.tile([128, HK], F32R, name=f"wo{b}_{o2}_{n}", tag="wo")
                            col_dma(wo_sb, wo_ap[b, :, o2 * 128:(o2 + 1) * 128], NO)
                            ps_y = bpsum.tile([128, NC_CHUNK], F32, name=f"psy{b}_{o2}_{n}", tag="psy")
                            for o in range(NO):
                                nc.tensor.matmul(
                                    ps_y[:], wo_sb[:, o * 128:(o + 1) * 128],
                                    att[o][:], start=(o == 0), stop=(o == NO - 1))
                            yt = bscr.tile([128, NC_CHUNK], F32, name=f"y_{b}_{o2}_{n}", tag="y")
                            nc.scalar.activation(yt[:], ps_y[:], AF.Gelu_apprx_tanh, bias=scol("bo", o2))
                            nc.vector.tensor_tensor(Z[o2][:, t0:t1], yt[:], X[o2][:, t0:t1], ALU.add)

                if debug and b == 0:
                    nc.sync.dma_start(dbg_aps["d_z1"][:], Z[0][:].bitcast(F32))

                # ---------------- LN1: x = LN(z) --------------------------
                _layer_norm(nc, tc, Z, X, sc_sb, "ln1", b, eps_col, ones_col, ones_row,
                            mybir, resid=None)

                if debug and b == 0:
                    nc.sync.dma_start(dbg_aps["d_ln1"][:], X[0][:].bitcast(F32))

                # ---------------- FFN: z2 = gelu(gelu(x@W1+b1)@W2+b2) + x --
                NGRP = 8
                GS = NF // NGRP  # 4 o3-tiles per group
                NCH = 4          # 512-token chunks for W1
                CH = TPC // NCH
                with (
                    tc.tile_pool(name="w1p", bufs=GS + 2) as w1p,
                    tc.tile_pool(name="w2p", bufs=GS + 2) as w2p,
                    tc.tile_pool(name="hp", bufs=GS + 2) as hp,
                    tc.tile_pool(name="fscr", bufs=2) as fscr,
                    tc.tile_pool(name="fps", bufs=2, space="PSUM") as fps,
                    tc.tile_pool(name="fps2", bufs=2, space="PSUM") as fps2,
                ):
                    for g in range(NGRP):
                        w1g, w2g = [], []
                        for j in range(GS):
                            o3 = g * GS + j
                            w1t = w1p.tile([128, D], F32R, name=f"w1_{b}_{o3}", tag="w1")
                            col_dma(w1t, w1_ap[b, :, o3 * 128:(o3 + 1) * 128], ND)
                            w2t = w2p.tile([128, D], F32R, name=f"w2_{b}_{o3}", tag="w2")
                            # W2 rows are already [f-part, d-free]: straight DMA
                            nc.gpsimd.dma_start(w2t[:], w2_ap[b, o3 * 128:(o3 + 1) * 128, :])
                            w1g.append(w1t)
                            w2g.append(w2t)
                        for c in range(NCH):
                            t0, t1 = c * CH, (c + 1) * CH
                            hg = []
                            for j in range(GS):
                                o3 = g * GS + j
                                ps_h = fps.tile([128, CH], F32, name=f"psh{b}_{o3}_{c}", tag="psh")
                                for d in range(ND):
                                    nc.tensor.matmul(
                                        ps_h[:], w1g[j][:, d * 128:(d + 1) * 128],
                                        X[d][:, t0:t1], start=(d == 0), stop=(d == ND - 1))
                                ht = hp.tile([128, CH], F32R, name=f"h_{b}_{o3}_{c}", tag="h")
                                nc.scalar.activation(ht[:], ps_h[:], AF.Gelu_apprx_tanh,
                                                     bias=scol("b1", o3))
                                if debug and b == 0 and o3 == 0 and c == 0:
                                    nc.sync.dma_start(dbg_aps["d_h0"][:], ht[:].bitcast(F32))
                                hg.append(ht)
                            HH = CH // 2
                            for half in range(2):
                                h0, h1 = half * HH, (half + 1) * HH
                                for o4h in range(2):
                                    o4s = o4h * (ND // 2)
                                    ps2 = fps2.tile([128, (ND // 2) * HH], F32,
                                                    name=f"ps2_{b}_{g}_{c}_{half}_{o4h}", tag="ps2")
                                    for oj in range(ND // 2):
                                        for j in range(GS):
                                            nc.tensor.matmul(
                                                ps2[:, oj * HH:(oj + 1) * HH],
                                                w2g[j][:, (o4s + oj) * 128:(o4s + oj + 1) * 128],
                                                hg[j][:, h0:h1],
                                                start=(j == 0), stop=(j == GS - 1))
                                    # spill/accumulate into Z (pre-gelu h2 partials)
                                    for oj in range(ND // 2):
                                        zsl = Z[o4s + oj][:, t0 + h0:t0 + h1]
                                        psl = ps2[:, oj * HH:(oj + 1) * HH]
                                        if g == 0:
                                            nc.scalar.activation(zsl, psl, AF.Copy)
                                        else:
                                            nc.vector.tensor_tensor(zsl, zsl, psl, ALU.add)
                    if debug and b == 0:
                        nc.sync.dma_start(dbg_aps["d_zacc"][:], Z[0][:].bitcast(F32))
                    # finalize: z2 = gelu(z2acc + b2) + x
                    for o4 in range(ND):
                        for c in range(NCH):
                            t0, t1 = c * CH, (c + 1) * CH
                            gt = fscr.tile([128, CH], F32, name=f"g2_{b}_{o4}_{c}", tag="g2")
                            nc.scalar.activation(gt[:], Z[o4][:, t0:t1],
                                                 AF.Gelu_apprx_tanh, bias=scol("b2", o4))
                            nc.vector.tensor_tensor(Z[o4][:, t0:t1], gt[:],
                                                    X[o4][:, t0:t1], ALU.add)

                if debug and b == 0:
                    nc.sync.dma_start(dbg_aps["d_ffnz"][:], Z[0][:].bitcast(F32))

                # ---------------- LN2: x = LN(z2) -------------------------
                _layer_norm(nc, tc, Z, X, sc_sb, "ln2", b, eps_col, ones_col, ones_row,
                            mybir, resid=None)

            # write result
            for d in range(ND):
                nc.sync.dma_start(out_ap[d * 128:(d + 1) * 128, :],
                                  X[d][:].bitcast(mybir.dt.float32))

    nc.compile()
    return nc


def _layer_norm(nc, tc, Zt, Xt, sc_sb, pref, b, eps_col, ones_col, ones_row, mybir,
                resid):
    """x = (z - mean)/sqrt(var+eps) * s + bias, feature-major.
    Stats over the partition (feature) axis via thin f32r matmuls;
    per-token values broadcast across partitions via K=1 matmuls."""
    F32 = mybir.dt.float32
    F32R = mybir.dt.float32r
    AF = mybir.ActivationFunctionType
    ALU = mybir.AluOpType
    NCH = 4
    CH = TPC // NCH
    s_c = _COLS[pref + "_s"]
    b_c = _COLS[pref + "_b"]
    with (
        tc.tile_pool(name=f"{pref}scr", bufs=2) as lscr,
        tc.tile_pool(name=f"{pref}ps", bufs=2, space="PSUM") as lps,
        tc.tile_pool(name=f"{pref}bc", bufs=2, space="PSUM") as lbc,
    ):
        for c in range(NCH):
            t0, t1 = c * CH, (c + 1) * CH
            ps_sum = lps.tile([1, CH], F32, name=f"{pref}sum{b}_{c}", tag="pssum")
            for o in range(ND):
                nc.tensor.matmul(ps_sum[:], ones_col[:], Zt[o][:, t0:t1],
                                 start=(o == 0), stop=(o == ND - 1))
            mrow = lscr.tile([1, CH], F32R, name=f"{pref}mr{b}_{c}", tag="mrow")
            nc.scalar.activation(mrow[:], ps_sum[:], AF.Copy)
            ps_sq = lps.tile([1, CH], F32, name=f"{pref}sq{b}_{c}", tag="pssq")
            for o in range(ND):
                zsq = lscr.tile([128, CH], F32R, name=f"{pref}zsq{b}_{c}_{o}", tag="zsq")
                nc.vector.tensor_tensor(zsq[:], Zt[o][:, t0:t1], Zt[o][:, t0:t1], ALU.mult)
                nc.tensor.matmul(ps_sq[:], ones_col[:], zsq[:],
                                 start=(o == 0), stop=(o == ND - 1))
            sqrow = lscr.tile([1, CH], F32R, name=f"{pref}sqr{b}_{c}", tag="sqrow")
            nc.scalar.activation(sqrow[:], ps_sq[:], AF.Copy)
            # broadcast mean and meansq across partitions
            bc_m = lbc.tile([128, CH], F32, name=f"{pref}bcm{b}_{c}", tag="bcm")
            nc.tensor.matmul(bc_m[:], ones_row[:], mrow[:], start=True, stop=True)
            bc_q = lbc.tile([128, CH], F32, name=f"{pref}bcq{b}_{c}", tag="bcq")
            nc.tensor.matmul(bc_q[:], ones_row[:], sqrow[:], start=True, stop=True)
            mean_b = lscr.tile([128, CH], F32, name=f"{pref}mb{b}_{c}", tag="meanb")
            nc.scalar.activation(mean_b[:], bc_m[:], AF.Copy)
            msq = lscr.tile([128, CH], F32, name=f"{pref}ms{b}_{c}", tag="msq")
            nc.scalar.activation(msq[:], bc_m[:], AF.Square)
            var = lscr.tile([128, CH], F32, name=f"{pref}var{b}_{c}", tag="var")
            nc.vector.tensor_tensor(var[:], bc_q[:], msq[:], ALU.subtract)
            std = lscr.tile([128, CH], F32, name=f"{pref}std{b}_{c}", tag="std")
            nc.scalar.activation(std[:], var[:], AF.Sqrt, bias=eps_col[:])
            rstd = lscr.tile([128, CH], F32, name=f"{pref}rstd{b}_{c}", tag="rstd")
            nc.vector.reciprocal_approx_fast(rstd[:], std[:])
            for o in range(ND):
                d0 = lscr.tile([128, CH], F32, name=f"{pref}d0{b}_{c}_{o}", tag="d0")
                nc.vector.tensor_tensor(d0[:], Zt[o][:, t0:t1], mean_b[:], ALU.subtract)
                d1 = lscr.tile([128, CH], F32, name=f"{pref}d1{b}_{c}_{o}", tag="d1")
                nc.vector.tensor_tensor(d1[:], d0[:], rstd[:], ALU.mult)
                nc.vector.tensor_scalar(Xt[o][:, t0:t1], d1[:],
                                        sc_sb[:, s_c + o:s_c + o + 1],
                                        sc_sb[:, b_c + o:b_c + o + 1],
                                        ALU.mult, ALU.add)


def _prep_inputs(inputs, n_blocks):
    """Host-side: shard x over sequence, pre-transpose to feature-major,
    pack weights/biases."""
    x = np.ascontiguousarray(np.asarray(inputs["x"], dtype=np.float32))
    Wq = np.asarray(inputs["Wq"], dtype=np.float32)
    Wk = np.asarray(inputs["Wk"], dtype=np.float32)
    Wv = np.asarray(inputs["Wv"], dtype=np.float32)
    Wo = np.asarray(inputs["Wo"], dtype=np.float32)
    W1 = np.asarray(inputs["W1"], dtype=np.float32)
    W2 = np.asarray(inputs["W2"], dtype=np.float32)

    def mk(name_arr):  # [B, H, D, K] -> [B, D, HK]
        return np.ascontiguousarray(
            name_arr.transpose(0, 2, 1, 3).reshape(B, D, HK)[:n_blocks])

    wq = mk(Wq)
    wk = mk(Wk)
    wv = mk(Wv)
    wo = np.ascontiguousarray(Wo[:n_blocks])
    w1 = np.ascontiguousarray(W1[:n_blocks])
    w2 = np.ascontiguousarray(W2[:n_blocks])

    scal = np.zeros((n_blocks, 128, NSCAL), np.float32)

    def put(name, arr2d):  # arr2d [n_blocks, width] -> 128-chunk columns
        w = arr2d.shape[1]
        ncol = w // 128
        scal[:, :, _COLS[name]:_COLS[name] + ncol] = \
            arr2d.reshape(n_blocks, ncol, 128).transpose(0, 2, 1)

    put("bq", np.asarray(inputs["bq"], np.float32).reshape(B, HK)[:n_blocks])
    put("bk", np.asarray(inputs["bk"], np.float32).reshape(B, HK)[:n_blocks])
    put("bv", np.asarray(inputs["bv"], np.float32).reshape(B, HK)[:n_blocks])
    put("bo", np.asarray(inputs["bo"], np.float32)[:n_blocks])
    put("b1", np.asarray(inputs["b1"], np.float32)[:n_blocks])
    put("b2", np.asarray(inputs["b2"], np.float32)[:n_blocks])
    put("ln1_s", np.asarray(inputs["ln1_s"], np.float32)[:n_blocks])
    put("ln1_b", np.asarray(inputs["ln1_b"], np.float32)[:n_blocks])
    put("ln2_s", np.asarray(inputs["ln2_s"], np.float32)[:n_blocks])
    put("ln2_b", np.asarray(inputs["ln2_b"], np.float32)[:n_blocks])

    in_maps = []
    for core in range(N_CORES):
        s0 = core * SEQ_SH
        # [N, SEQ_SH, D] -> [D, N*SEQ_SH] feature-major
        xc = np.ascontiguousarray(
            x[:, s0:s0 + SEQ_SH, :].transpose(2, 0, 1).reshape(D, TPC))
        in_maps.append({
            "x_fm": xc, "wq": wq, "wk": wk, "wv": wv, "wo": wo,
            "w1": w1, "w2": w2, "scal": scal,
        })
    return in_maps


def run(inputs, n_blocks=B, trace=False, debug=False):
    from concourse.bass_utils import run_bass_kernel_spmd

    key = (n_blocks, debug)
    if key not in _BUILD_CACHE:
        _BUILD_CACHE[key] = _build(n_blocks, debug=debug)
    nc = _BUILD_CACHE[key]
    in_maps = _prep_inputs(inputs, n_blocks)
    res = run_bass_kernel_spmd(nc, in_maps, list(range(N_CORES)), trace=trace)
    # gather: per-core [D, TPC] feature-major -> [N, S, D]
    out = np.empty((N, S, D), np.float32)
    for core in range(N_CORES):
        s0 = core * SEQ_SH
        oc = res.results[core]["out_fm"]  # [D, TPC]
        out[:, s0:s0 + SEQ_SH, :] = \
            oc.reshape(D, N, SEQ_SH).transpose(1, 2, 0)
    return out, res


def kernel(**inputs):
    out, _ = run(inputs, n_blocks=B, trace=False)
    return out



# revision 14
# speedup vs baseline: 1.0353x; 1.0353x over previous
"""Trainium2 Bass kernel for an 8-block linear-attention transformer.

Contract: kernel(**inputs) takes full unsharded inputs (as in
reference.setup_inputs()) and returns the full [N, S, D] output.

Sharding: sequence-parallel over the 16384 tokens -> 2048 tokens/core on
8 NeuronCores. The only cross-token coupling is the per-sample KV/sumK
sums of the linear attention; each core computes partial sums over its
local tokens and a tiny [128, 64] fp32 AllReduce per block produces the
global sums. Everything else is purely token-parallel.

v3 design notes:
- Matmul inputs bf16 (weights + X + att + h); fp32r for the Z residual
  master, LN internals and gelu outputs feeding Z. PSUM/stats fp32.
- LayerNorm: stats matmuls use a full [128,128] ones/D stationary so the
  8-matmul accumulation lands the broadcast mean/E[z^2] tiles directly
  (no thin M=1 matmuls, no separate broadcast matmuls, no row chains);
  rstd via one wide Abs_reciprocal_sqrt; apply is two tensor_tensor ops
  with the subtraction done in place on the dying Z tile.
- FFN second matmul accumulates all 32 f-tiles in PSUM per chunk.
- scalar_tensor_tensor runs 1x on DVE regardless of dtype, so hot
  elementwise ops are tensor_tensor (2x bf16) where possible; the
  attention 1/den is folded into the PSUM->SBUF ACT copy via its
  per-partition scale operand.
- ACT table sets: exp_and_others -> gelu_apprx_tanh -> abs_r_sqrt,
  ~5 loads per block.
- All Q projections for a half of the token chunks are issued before
  anything depending on the AllReduce, hiding the collective.
"""

import sys

sys.path.insert(0, "/opt/trn_rl_repo")

import numpy as np

# dims (hardcoded; must match reference.py)
B, H, D, K, F = 8, 16, 1024, 64, 4096
N, S = 4, 4096
HK = H * K  # 1024
N_CORES = 8
TPC = (N * S) // N_CORES  # tokens per core = 2048
SEQ_SH = S // N_CORES     # 512 local tokens per sample
NC_CHUNK = TPC // N       # 512 = one sample's local tokens (= chunk)
ND = D // 128             # 8 d-tiles
NO = HK // 128            # 8 attention-feature tiles
NF = F // 128             # 32 ffn tiles
LN_EPS = 1e-6
ATT_EPS = 1e-6

# bias/scale column layout in the packed per-block fp32 scalar tensor
_COLS = {}
_c = 0
for _name, _n in [("bq", NO), ("bk", NO), ("bv", NO), ("nbq", NO), ("nbk", NO),
                  ("bo", ND), ("ln1_s", ND), ("ln1_b", ND),
                  ("ln2_s", ND), ("ln2_b", ND), ("b2", ND), ("b1", NF)]:
    _COLS[_name] = _c
    _c += _n
NSCAL = _c

_BUILD_CACHE = {}


def _build(n_blocks, ln_affine=False, ln_bias=False, debug=False):
    """Build the SPMD bass program (same program for all 8 cores)."""
    import concourse.bacc as bacc
    import concourse.tile as tile
    import concourse.mybir as mybir

    F32 = mybir.dt.float32
    F32R = mybir.dt.float32r
    BF16 = mybir.dt.bfloat16
    AF = mybir.ActivationFunctionType
    ALU = mybir.AluOpType

    nc = bacc.Bacc("TRN2", target_bir_lowering=False, debug=False,
                   num_devices=N_CORES)

    x_ap = nc.dram_tensor("x_fm", [D, TPC], BF16, kind="ExternalInput").ap()
    wkv_ap = nc.dram_tensor("wkv", [n_blocks, NO, 128, 2 * D], BF16, kind="ExternalInput").ap()
    wq_ap = nc.dram_tensor("wq", [n_blocks, NO, 128, D], BF16, kind="ExternalInput").ap()
    wo_ap = nc.dram_tensor("wo", [n_blocks, ND, 128, HK], BF16, kind="ExternalInput").ap()
    w1_ap = nc.dram_tensor("w1", [n_blocks, NF, 128, D], BF16, kind="ExternalInput").ap()
    # w2 packed per (o4, f-quarter): [n_blocks, ND, 4, 128, F/4]
    w2_ap = nc.dram_tensor("w2", [n_blocks, ND, 4, 128, F // 4], BF16, kind="ExternalInput").ap()
    sc_ap = nc.dram_tensor("scal", [n_blocks, 128, NSCAL], F32, kind="ExternalInput").ap()
    out_ap = nc.dram_tensor("out_fm", [D, TPC], F32, kind="ExternalOutput").ap()
    dbg_aps = {}
    if debug:
        for nm, shp, dt in [("d_kvs", [128, 64], F32), ("d_att0", [128, TPC], BF16),
                            ("d_z1", [128, TPC], F32), ("d_ln1", [128, TPC], BF16),
                            ("d_h0", [128, 512], BF16), ("d_ffnz", [128, TPC], F32)]:
            dbg_aps[nm] = nc.dram_tensor(nm, shp, dt, kind="ExternalOutput").ap()

    with tile.TileContext(nc) as tc:
        with (
            tc.tile_pool(name="resid", bufs=1) as resid,
            tc.tile_pool(name="consts", bufs=1) as consts,
            tc.tile_pool(name="scalp", bufs=2) as scalp,
            tc.tile_pool(name="kvsp", bufs=2) as kvsp,
            tc.tile_pool(name="wp", bufs=2) as wp,
            tc.tile_pool(name="hp", bufs=32) as hp,
            tc.tile_pool(name="qfp", bufs=3) as qfp,
            tc.tile_pool(name="bscr", bufs=4) as bscr,
            tc.tile_pool(name="fscr", bufs=3) as fscr,
            tc.tile_pool(name="scr", bufs=2) as scr,
            tc.tile_pool(name="actp", bufs=2) as actp,
            tc.tile_pool(name="bcp", bufs=2) as bcp,
            tc.tile_pool(name="ps", bufs=8, space="PSUM") as psp,
            tc.tile_pool(name="dram", bufs=2, space="DRAM") as dramp,
        ):
            # persistent residual-stream buffers (feature-major)
            X = [resid.tile([128, TPC], BF16, name=f"X{d}", tag=f"X{d}")
                 for d in range(ND)]
            Z = [resid.tile([128, TPC], F32R, name=f"Z{d}", tag=f"Z{d}")
                 for d in range(ND)]

            # constants
            ones_f = consts.tile([128, 128], F32)
            nc.vector.memset(ones_f[:], 1.0 / D)
            ones_sq = consts.tile([128, 128], F32R)  # 1/D stats+broadcast lhsT
            nc.scalar.activation(ones_sq[:], ones_f[:], AF.Copy)
            eps_col = consts.tile([128, 1], F32)
            nc.vector.memset(eps_col[:], LN_EPS)

            # load input activations; Z is the fp32 residual-stream master,
            # X the bf16 matmul mirror
            for d in range(ND):
                nc.gpsimd.dma_start(X[d][:], x_ap[d * 128:(d + 1) * 128, :])
                nc.scalar.activation(Z[d][:], X[d][:], AF.Copy)

            def ln(pref, b, scol, final=False):
                """z = LN(z) in place (fp32 master); x = bf16 copy of z for
                the matmuls. Stats matmuls with a [128,128] ones/D stationary
                produce broadcast mean / E[z^2] tiles directly in PSUM.
                For the last LN, DMA the fp32 result out instead of mirroring."""
                for c in range(N):
                    t0, t1 = c * NC_CHUNK, (c + 1) * NC_CHUNK
                    bc_m = psp.tile([128, NC_CHUNK], F32, name=f"{pref}bm{b}_{c}", tag="ps")
                    for o in range(ND):
                        nc.tensor.matmul(bc_m[:], ones_sq[:], Z[o][:, t0:t1],
                                         start=(o == 0), stop=(o == ND - 1))
                    bc_q = psp.tile([128, NC_CHUNK], F32, name=f"{pref}bq{b}_{c}", tag="ps")
                    for o in range(ND):
                        zsq = scr.tile([128, NC_CHUNK], F32R, name=f"{pref}zs{b}_{c}_{o}", tag="zsq")
                        nc.scalar.activation(zsq[:], Z[o][:, t0:t1], AF.Square)
                        nc.tensor.matmul(bc_q[:], ones_sq[:], zsq[:],
                                         start=(o == 0), stop=(o == ND - 1))
                    # var = E[z^2] - mean^2, in place on the mean-square tile
                    msq = bcp.tile([128, NC_CHUNK], F32, name=f"{pref}ms{b}_{c}", tag="msq")
                    nc.scalar.activation(msq[:], bc_m[:], AF.Square)
                    nc.vector.tensor_tensor(msq[:], bc_q[:], msq[:], ALU.subtract)
                    rstd = bcp.tile([128, NC_CHUNK], F32R, name=f"{pref}rs{b}_{c}", tag="rstd")
                    nc.scalar.activation(rstd[:], msq[:], AF.Abs_reciprocal_sqrt,
                                         bias=eps_col[:])
                    for o in range(ND):
                        zs = Z[o][:, t0:t1]
                        nc.vector.tensor_tensor(zs, zs, bc_m[:], ALU.subtract)
                        if not ln_affine:
                            nc.vector.tensor_tensor(zs, zs, rstd[:], ALU.mult)
                        else:
                            nc.vector.scalar_tensor_tensor(
                                zs, zs, scol(pref + "_s", o), rstd[:],
                                ALU.mult, ALU.mult)
                            if ln_bias:
                                nc.vector.tensor_scalar(
                                    zs, zs, scol(pref + "_b", o), None, ALU.add)
                        if final:
                            nc.sync.dma_start(
                                out_ap[o * 128:(o + 1) * 128, t0:t1],
                                zs.bitcast(F32))
                        else:
                            nc.vector.tensor_copy(X[o][:, t0:t1], zs)

            for b in range(n_blocks):
                sc_sb = scalp.tile([128, NSCAL], F32, name=f"sc{b}", tag="sc")
                nc.sync.dma_start(sc_sb[:], sc_ap[b])

                def scol(name, i):
                    return sc_sb[:, _COLS[name] + i:_COLS[name] + i + 1]

                # ---------------- Phase A: K, V -> local KV/sumK ----------
                kvs_loc = kvsp.tile([128, 2 * NO * N], F32, name=f"kvl{b}", tag="kvl")
                # col layout: kv[o][n] at o*8+n, sumk[o][n] at o*8+4+n
                for o in range(NO):
                    wkv_sb = wp.tile([128, 2 * D], BF16, name=f"wkv{b}_{o}", tag="wkv")
                    nc.gpsimd.dma_start(wkv_sb[:], wkv_ap[b, o])
                    for n in range(N):
                        t0, t1 = n * NC_CHUNK, (n + 1) * NC_CHUNK
                        ps_k = psp.tile([128, NC_CHUNK], F32, name=f"psk{b}_{o}_{n}", tag="ps")
                        for d in range(ND):
                            nc.tensor.matmul(
                                ps_k[:], wkv_sb[:, d * 128:(d + 1) * 128],
                                X[d][:, t0:t1], start=(d == 0), stop=(d == ND - 1))
                        ps_v = psp.tile([128, NC_CHUNK], F32, name=f"psv{b}_{o}_{n}", tag="ps")
                        for d in range(ND):
                            nc.tensor.matmul(
                                ps_v[:], wkv_sb[:, D + d * 128:D + (d + 1) * 128],
                                X[d][:, t0:t1], start=(d == 0), stop=(d == ND - 1))
                        # kf = exp(min(k+bk,0)) + relu(k+bk)
                        y1 = bscr.tile([128, NC_CHUNK], BF16, name=f"y1_{b}_{o}_{n}", tag="bscr")
                        nc.scalar.activation(y1[:], ps_k[:], AF.Relu,
                                             bias=scol("nbk", o), scale=-1.0)
                        t2 = bscr.tile([128, NC_CHUNK], BF16, name=f"t2_{b}_{o}_{n}", tag="bscr")
                        nc.scalar.activation(t2[:], ps_k[:], AF.Relu, bias=scol("bk", o))
                        nc.scalar.activation(y1[:], y1[:], AF.Exp, scale=-1.0)
                        vsb = scr.tile([128, NC_CHUNK], BF16, name=f"vs_{b}_{o}_{n}", tag="vsb")
                        nc.vector.tensor_copy(vsb[:], ps_v[:])
                        # fp32 out: accum_out precision follows the out dtype
                        kf = fscr.tile([128, NC_CHUNK], F32, name=f"kf_{b}_{o}_{n}", tag="fscr")
                        nc.vector.scalar_tensor_tensor(
                            kf[:], y1[:], 0.0, t2[:], ALU.add, ALU.add,
                            accum_out=kvs_loc[:, o * 8 + 4 + n:o * 8 + 5 + n])
                        nc.vector.scalar_tensor_tensor(
                            kf[:], kf[:], 1.0, vsb[:], ALU.mult, ALU.mult,
                            accum_out=kvs_loc[:, o * 8 + n:o * 8 + 1 + n])

                # ---------------- AllReduce of KV/sumK --------------------
                cc_in = dramp.tile([128, 2 * NO * N], F32, name=f"cci{b}", tag="cci")
                cc_out = dramp.tile([128, 2 * NO * N], F32, name=f"cco{b}", tag="cco")
                nc.sync.dma_start(cc_in[:], kvs_loc[:])
                nc.gpsimd.collective_compute(
                    "AllReduce", mybir.AluOpType.add,
                    replica_groups=[list(range(N_CORES))],
                    ins=[cc_in.opt()], outs=[cc_out.opt()])
                kvs = kvsp.tile([128, 2 * NO * N], F32, name=f"kvg{b}", tag="kvg")
                nc.sync.dma_start(kvs[:], cc_out[:])
                # fold V-bias into KV: kv += bv * sumk  (exact)
                for o in range(NO):
                    nc.vector.scalar_tensor_tensor(
                        kvs[:, o * 8:o * 8 + 4], kvs[:, o * 8 + 4:o * 8 + 8],
                        scol("bv", o), kvs[:, o * 8:o * 8 + 4], ALU.mult, ALU.add)
                if debug and b == 0:
                    nc.sync.dma_start(dbg_aps["d_kvs"][:], kvs[:])

                # ------------- Phase B, split in n-halves -----------------
                for nh in range(2):
                    ns = (2 * nh, 2 * nh + 1)
                    qf = {}
                    # B1: Q projections + feature map (independent of the CC)
                    for o in range(NO):
                        wq_sb = wp.tile([128, D], BF16, name=f"wq{b}_{o}_{nh}", tag="wq")
                        nc.gpsimd.dma_start(wq_sb[:], wq_ap[b, o])
                        for n in ns:
                            t0, t1 = n * NC_CHUNK, (n + 1) * NC_CHUNK
                            ps_q = psp.tile([128, NC_CHUNK], F32, name=f"psq{b}_{o}_{n}", tag="ps")
                            for d in range(ND):
                                nc.tensor.matmul(
                                    ps_q[:], wq_sb[:, d * 128:(d + 1) * 128],
                                    X[d][:, t0:t1], start=(d == 0), stop=(d == ND - 1))
                            y1q = bscr.tile([128, NC_CHUNK], BF16, name=f"yq_{b}_{o}_{n}", tag="bscr")
                            nc.scalar.activation(y1q[:], ps_q[:], AF.Relu,
                                                 bias=scol("nbq", o), scale=-1.0)
                            t2q = bscr.tile([128, NC_CHUNK], BF16, name=f"qt2_{b}_{o}_{n}", tag="bscr")
                            nc.scalar.activation(t2q[:], ps_q[:], AF.Relu, bias=scol("bq", o))
                            nc.scalar.activation(y1q[:], y1q[:], AF.Exp, scale=-1.0)
                            qt = qfp.tile([128, NC_CHUNK], BF16, name=f"qf{b}_{o}_{n}", tag=f"qf{o}")
                            nc.vector.tensor_tensor(qt[:], y1q[:], t2q[:], ALU.add)
                            qf[(o, n)] = qt

                    # B2: att = qf * kv / (qf*sumk + eps), in place on qf
                    for n in ns:
                        for o in range(NO):
                            qt = qf[(o, n)]
                            den = fscr.tile([128, NC_CHUNK], F32, name=f"dn_{b}_{o}_{n}", tag="fscr")
                            nc.vector.tensor_scalar(
                                den[:], qt[:], kvs[:, o * 8 + 4 + n:o * 8 + 5 + n],
                                ATT_EPS, ALU.mult, ALU.add)
                            nc.vector.reciprocal_approx_fast(den[:], den[:])
                            # recb = kv / den via the ACT per-partition scale
                            recb = bscr.tile([128, NC_CHUNK], BF16, name=f"rb_{b}_{o}_{n}", tag="bscr")
                            nc.scalar.activation(recb[:], den[:], AF.Copy,
                                                 scale=kvs[:, o * 8 + n:o * 8 + 1 + n])
                            nc.vector.tensor_tensor(qt[:], qt[:], recb[:], ALU.mult)
                            if debug and b == 0 and o == 0:
                                t0, t1 = n * NC_CHUNK, (n + 1) * NC_CHUNK
                                nc.sync.dma_start(dbg_aps["d_att0"][:, t0:t1], qt[:])

                    # B3: Wo -> gelu -> z = y + x
                    for o2 in range(ND):
                        wo_sb = wp.tile([128, HK], BF16, name=f"wo{b}_{o2}_{nh}", tag="wo")
                        nc.gpsimd.dma_start(wo_sb[:], wo_ap[b, o2])
                        for n in ns:
                            t0, t1 = n * NC_CHUNK, (n + 1) * NC_CHUNK
                            ps_y = psp.tile([128, NC_CHUNK], F32, name=f"psy{b}_{o2}_{n}", tag="ps")
                            for o in range(NO):
                                nc.tensor.matmul(
                                    ps_y[:], wo_sb[:, o * 128:(o + 1) * 128],
                                    qf[(o, n)][:], start=(o == 0), stop=(o == NO - 1))
                            yt = actp.tile([128, NC_CHUNK], F32R, name=f"y_{b}_{o2}_{n}", tag="actout")
                            nc.scalar.activation(yt[:], ps_y[:], AF.Gelu_apprx_tanh,
                                                 bias=scol("bo", o2))
                            nc.vector.tensor_tensor(Z[o2][:, t0:t1], yt[:], Z[o2][:, t0:t1],
                                                    ALU.add)

                if debug and b == 0:
                    nc.sync.dma_start(dbg_aps["d_z1"][:], Z[0][:].bitcast(F32))

                # ---------------- LN1: x = LN(z) --------------------------
                ln("ln1", b, scol)

                if debug and b == 0:
                    nc.sync.dma_start(dbg_aps["d_ln1"][:], X[0][:])

                # ------- FFN: z2 = gelu(gelu(x@W1+b1)@W2+b2) + x ----------
                for c in range(N):
                    t0, t1 = c * NC_CHUNK, (c + 1) * NC_CHUNK
                    hs = []
                    for f in range(NF):
                        w1t = wp.tile([128, D], BF16, name=f"w1_{b}_{c}_{f}", tag="w1")
                        nc.sync.dma_start(w1t[:], w1_ap[b, f])
                        ps_h = psp.tile([128, NC_CHUNK], F32, name=f"psh{b}_{c}_{f}", tag="ps")
                        for d in range(ND):
                            nc.tensor.matmul(
                                ps_h[:], w1t[:, d * 128:(d + 1) * 128],
                                X[d][:, t0:t1], start=(d == 0), stop=(d == ND - 1))
                        ht = hp.tile([128, NC_CHUNK], BF16, name=f"h_{b}_{c}_{f}", tag="h")
                        nc.scalar.activation(ht[:], ps_h[:], AF.Gelu_apprx_tanh,
                                             bias=scol("b1", f))
                        if debug and b == 0 and f == 0 and c == 0:
                            nc.sync.dma_start(dbg_aps["d_h0"][:], ht[:])
                        hs.append(ht)
                    for o4 in range(ND):
                        ps2 = psp.tile([128, NC_CHUNK], F32, name=f"ps2{b}_{c}_{o4}", tag="ps")
                        for quart in range(4):
                            w2t = wp.tile([128, F // 4], BF16,
                                          name=f"w2_{b}_{c}_{o4}_{quart}", tag="w2")
                            nc.gpsimd.dma_start(w2t[:], w2_ap[b, o4, quart])
                            for j in range(NF // 4):
                                f = quart * (NF // 4) + j
                                nc.tensor.matmul(
                                    ps2[:], w2t[:, j * 128:(j + 1) * 128], hs[f][:],
                                    start=(f == 0), stop=(f == NF - 1))
                        gt = actp.tile([128, NC_CHUNK], F32R, name=f"g_{b}_{c}_{o4}", tag="actout")
                        nc.scalar.activation(gt[:], ps2[:], AF.Gelu_apprx_tanh,
                                             bias=scol("b2", o4))
                        nc.vector.tensor_tensor(Z[o4][:, t0:t1], gt[:], Z[o4][:, t0:t1],
                                                ALU.add)

                if debug and b == 0:
                    nc.sync.dma_start(dbg_aps["d_ffnz"][:], Z[0][:].bitcast(F32))

                # ---------------- LN2: x = LN(z2) -------------------------
                ln("ln2", b, scol, final=(b == n_blocks - 1))

    nc.compile()
    return nc


def _prep_inputs(inputs, n_blocks):
    """Host-side: shard x over sequence, pre-transpose to feature-major
    bf16, pack weights as contiguous lhsT tiles, pack biases/scales."""
    import ml_dtypes

    bf16 = ml_dtypes.bfloat16
    x = np.asarray(inputs["x"], dtype=np.float32)
    Wq = np.asarray(inputs["Wq"], dtype=np.float32)
    Wk = np.asarray(inputs["Wk"], dtype=np.float32)
    Wv = np.asarray(inputs["Wv"], dtype=np.float32)
    Wo = np.asarray(inputs["Wo"], dtype=np.float32)
    W1 = np.asarray(inputs["W1"], dtype=np.float32)
    W2 = np.asarray(inputs["W2"], dtype=np.float32)

    def qkv_pack(arr):  # [B,H,D,K] -> [B,D,HK] -> [nb, o, p(d), dd, m(hk)]
        a2 = arr.transpose(0, 2, 1, 3).reshape(B, D, HK)[:n_blocks]
        a5 = a2.reshape(n_blocks, ND, 128, NO, 128)     # [b, dd, p(d), o, m(hk)]
        # lhsT tile[p(d in dd), dd*128+m(hk)] = W[b, d=dd*128+p, hk=o*128+m]
        return a5.transpose(0, 3, 2, 1, 4).astype(bf16)  # [b, o, p, dd, m]

    wq = np.ascontiguousarray(qkv_pack(Wq))
    wkv = np.ascontiguousarray(
        np.concatenate([qkv_pack(Wk).reshape(n_blocks, NO, 128, D),
                        qkv_pack(Wv).reshape(n_blocks, NO, 128, D)], axis=3))
    # Wo [B, HK, D]: tile[o2][p(hk in o), o*128+m(d in o2)] = Wo[b, o*128+p, o2*128+m]
    wo = np.ascontiguousarray(
        Wo[:n_blocks].reshape(n_blocks, NO, 128, ND, 128)
        .transpose(0, 3, 2, 1, 4).astype(bf16))          # [b, o2, p, o, m]
    # W1 [B, D, F]: tile[f][p(d in dd), dd*128+m(f)] = W1[b, d=dd*128+p, f=f*128+m]
    w1 = np.ascontiguousarray(
        W1[:n_blocks].reshape(n_blocks, ND, 128, NF, 128)
        .transpose(0, 3, 2, 1, 4).astype(bf16))          # [b, f, p, dd, m]
    # W2 [B, F, D]: tile[o4][p(f in ft), ft*128+m(d in o4)] = W2[b, ft*128+p, o4*128+m]
    w2 = np.ascontiguousarray(
        W2[:n_blocks].reshape(n_blocks, NF, 128, ND, 128)
        .transpose(0, 3, 2, 1, 4)                        # [b, o4, p, ft, m]
        .reshape(n_blocks, ND, 128, 4, (NF // 4) * 128)  # split f-quarters
        .transpose(0, 1, 3, 2, 4).astype(bf16))          # [b, o4, quart, p, cols]

    scal = np.zeros((n_blocks, 128, NSCAL), np.float32)

    def put(name, arr2d):  # arr2d [n_blocks, width] -> 128-chunk columns
        w = arr2d.shape[1]
        ncol = w // 128
        scal[:, :, _COLS[name]:_COLS[name] + ncol] = \
            arr2d.reshape(n_blocks, ncol, 128).transpose(0, 2, 1)

    bq2 = np.asarray(inputs["bq"], np.float32).reshape(B, HK)[:n_blocks]
    bk2 = np.asarray(inputs["bk"], np.float32).reshape(B, HK)[:n_blocks]
    put("bq", bq2)
    put("bk", bk2)
    put("nbq", -bq2)
    put("nbk", -bk2)
    put("bv", np.asarray(inputs["bv"], np.float32).reshape(B, HK)[:n_blocks])
    put("bo", np.asarray(inputs["bo"], np.float32)[:n_blocks])
    put("b1", np.asarray(inputs["b1"], np.float32)[:n_blocks])
    put("b2", np.asarray(inputs["b2"], np.float32)[:n_blocks])
    put("ln1_s", np.asarray(inputs["ln1_s"], np.float32)[:n_blocks])
    put("ln1_b", np.asarray(inputs["ln1_b"], np.float32)[:n_blocks])
    put("ln2_s", np.asarray(inputs["ln2_s"], np.float32)[:n_blocks])
    put("ln2_b", np.asarray(inputs["ln2_b"], np.float32)[:n_blocks])

    ln_bias = bool(
        np.any(np.asarray(inputs["ln1_b"])[:n_blocks]) or
        np.any(np.asarray(inputs["ln2_b"])[:n_blocks]))
    ln_affine = ln_bias or bool(
        np.any(np.asarray(inputs["ln1_s"])[:n_blocks] != 1.0) or
        np.any(np.asarray(inputs["ln2_s"])[:n_blocks] != 1.0))

    in_maps = []
    for core in range(N_CORES):
        s0 = core * SEQ_SH
        xc = np.ascontiguousarray(
            x[:, s0:s0 + SEQ_SH, :].transpose(2, 0, 1).reshape(D, TPC).astype(bf16))
        in_maps.append({
            "x_fm": xc, "wkv": wkv, "wq": wq, "wo": wo,
            "w1": w1, "w2": w2, "scal": scal,
        })
    return in_maps, ln_affine, ln_bias


def run(inputs, n_blocks=B, trace=False, debug=False):
    from concourse.bass_utils import run_bass_kernel_spmd

    in_maps, ln_affine, ln_bias = _prep_inputs(inputs, n_blocks)
    key = (n_blocks, ln_affine, ln_bias, debug)
    if key not in _BUILD_CACHE:
        _BUILD_CACHE[key] = _build(n_blocks, ln_affine=ln_affine,
                                   ln_bias=ln_bias, debug=debug)
    nc = _BUILD_CACHE[key]
    res = run_bass_kernel_spmd(nc, in_maps, list(range(N_CORES)), trace=trace)
    # gather: per-core [D, TPC] feature-major -> [N, S, D]
    out = np.empty((N, S, D), np.float32)
    for core in range(N_CORES):
        s0 = core * SEQ_SH
        oc = np.asarray(res.results[core]["out_fm"]).astype(np.float32)  # [D, TPC]
        out[:, s0:s0 + SEQ_SH, :] = \
            oc.reshape(D, N, SEQ_SH).transpose(1, 2, 0)
    return out, res


def kernel(**inputs):
    out, _ = run(inputs, n_blocks=B, trace=False)
    return out


# revision 22
# speedup vs baseline: 1.0992x; 1.0618x over previous
"""Trainium2 Bass kernel for an 8-block linear-attention transformer.

Contract: kernel(**inputs) takes full unsharded inputs (as in
reference.setup_inputs()) and returns the full [N, S, D] output.

Sharding: sequence-parallel over the 16384 tokens -> 2048 tokens/core on
8 NeuronCores. The only cross-token coupling is the per-sample KV/sumK
sums of the linear attention; each core computes partial sums over its
local tokens and a tiny [128, 64] fp32 AllReduce per block produces the
global sums. Everything else is purely token-parallel.

v3 design notes:
- Matmul inputs bf16 (weights + X + att + h); fp32r for the Z residual
  master, LN internals and gelu outputs feeding Z. PSUM/stats fp32.
- LayerNorm: stats matmuls use a full [128,128] ones/D stationary so the
  8-matmul accumulation lands the broadcast mean/E[z^2] tiles directly
  (no thin M=1 matmuls, no separate broadcast matmuls, no row chains);
  rstd via one wide Abs_reciprocal_sqrt; apply is two tensor_tensor ops
  with the subtraction done in place on the dying Z tile.
- FFN second matmul accumulates all 32 f-tiles in PSUM per chunk.
- scalar_tensor_tensor runs 1x on DVE regardless of dtype, so hot
  elementwise ops are tensor_tensor (2x bf16) where possible; the
  attention 1/den is folded into the PSUM->SBUF ACT copy via its
  per-partition scale operand.
- ACT table sets: exp_and_others -> gelu_apprx_tanh -> abs_r_sqrt,
  ~5 loads per block.
- All Q projections for a half of the token chunks are issued before
  anything depending on the AllReduce, hiding the collective.
"""

import sys

sys.path.insert(0, "/opt/trn_rl_repo")

import numpy as np

# dims (hardcoded; must match reference.py)
B, H, D, K, F = 8, 16, 1024, 64, 4096
N, S = 4, 4096
HK = H * K  # 1024
N_CORES = 8
TPC = (N * S) // N_CORES  # tokens per core = 2048
SEQ_SH = S // N_CORES     # 512 local tokens per sample
NC_CHUNK = TPC // N       # 512 = one sample's local tokens (= chunk)
ND = D // 128             # 8 d-tiles
NO = HK // 128            # 8 attention-feature tiles
NF = F // 128             # 32 ffn tiles
LN_EPS = 1e-6
ATT_EPS = 1e-6

# bias/scale column layout in the packed per-block fp32 scalar tensor
_COLS = {}
_c = 0
for _name, _n in [("bq", NO), ("bk", NO), ("bv", NO), ("nbq", NO), ("nbk", NO),
                  ("bo", ND), ("ln1_s", ND), ("ln1_b", ND),
                  ("ln2_s", ND), ("ln2_b", ND), ("b2", ND), ("b1", NF)]:
    _COLS[_name] = _c
    _c += _n
NSCAL = _c

_BUILD_CACHE = {}


def _build(n_blocks, ln_affine=False, ln_bias=False, debug=False):
    """Build the SPMD bass program (same program for all 8 cores)."""
    import concourse.bacc as bacc
    import concourse.tile as tile
    import concourse.mybir as mybir

    F32 = mybir.dt.float32
    F32R = mybir.dt.float32r
    BF16 = mybir.dt.bfloat16
    AF = mybir.ActivationFunctionType
    ALU = mybir.AluOpType

    nc = bacc.Bacc("TRN2", target_bir_lowering=False, debug=False,
                   num_devices=N_CORES)

    x_ap = nc.dram_tensor("x_fm", [D, TPC], BF16, kind="ExternalInput").ap()
    wkv_ap = nc.dram_tensor("wkv", [n_blocks, NO, 128, 2 * D], BF16, kind="ExternalInput").ap()
    wq_ap = nc.dram_tensor("wq", [n_blocks, NO, 128, D], BF16, kind="ExternalInput").ap()
    wo_ap = nc.dram_tensor("wo", [n_blocks, ND, 128, HK], BF16, kind="ExternalInput").ap()
    w1_ap = nc.dram_tensor("w1", [n_blocks, NF, 128, D], BF16, kind="ExternalInput").ap()
    # w2 packed per (o4, f-quarter): [n_blocks, ND, 4, 128, F/4]
    w2_ap = nc.dram_tensor("w2", [n_blocks, ND, 4, 128, F // 4], BF16, kind="ExternalInput").ap()
    sc_ap = nc.dram_tensor("scal", [n_blocks, 128, NSCAL], F32, kind="ExternalInput").ap()
    out_ap = nc.dram_tensor("out_fm", [D, TPC], F32, kind="ExternalOutput").ap()
    dbg_aps = {}
    if debug:
        for nm, shp, dt in [("d_kvs", [128, 64], F32), ("d_att0", [128, TPC], BF16),
                            ("d_z1", [128, TPC], F32), ("d_ln1", [128, TPC], BF16),
                            ("d_h0", [128, 512], BF16), ("d_ffnz", [128, TPC], F32)]:
            dbg_aps[nm] = nc.dram_tensor(nm, shp, dt, kind="ExternalOutput").ap()

    with tile.TileContext(nc) as tc:
        with (
            tc.tile_pool(name="resid", bufs=1) as resid,
            tc.tile_pool(name="consts", bufs=1) as consts,
            tc.tile_pool(name="scalp", bufs=1) as scalp,
            tc.tile_pool(name="kvsp", bufs=2) as kvsp,
            tc.tile_pool(name="wp", bufs=2) as wp,
            tc.tile_pool(name="w1p", bufs=4) as w1p,
            tc.tile_pool(name="hp", bufs=32) as hp,
            tc.tile_pool(name="qfp", bufs=3) as qfp,
            tc.tile_pool(name="bscr", bufs=3) as bscr,
            tc.tile_pool(name="fscr", bufs=2) as fscr,
            tc.tile_pool(name="scr", bufs=2) as scr,
            tc.tile_pool(name="actp", bufs=2) as actp,
            tc.tile_pool(name="bcp", bufs=2) as bcp,
            tc.tile_pool(name="ps", bufs=8, space="PSUM") as psp,
            tc.tile_pool(name="dram", bufs=2, space="DRAM") as dramp,
        ):
            # persistent residual-stream buffers (feature-major)
            X = [resid.tile([128, TPC], BF16, name=f"X{d}", tag=f"X{d}")
                 for d in range(ND)]
            Z = [resid.tile([128, TPC], F32R, name=f"Z{d}", tag=f"Z{d}")
                 for d in range(ND)]

            # constants
            ones_f = consts.tile([128, 128], F32)
            nc.vector.memset(ones_f[:], 1.0 / D)
            ones_sq = consts.tile([128, 128], F32R)  # 1/D stats+broadcast lhsT
            nc.scalar.activation(ones_sq[:], ones_f[:], AF.Copy)
            eps_col = consts.tile([128, 1], F32)
            nc.vector.memset(eps_col[:], LN_EPS)

            # load input activations; Z is the fp32 residual-stream master,
            # X the bf16 matmul mirror
            for d in range(ND):
                nc.gpsimd.dma_start(X[d][:], x_ap[d * 128:(d + 1) * 128, :])
                nc.scalar.activation(Z[d][:], X[d][:], AF.Copy)

            def ln(pref, b, scol, final=False):
                """z = LN(z) in place (fp32 master); x = bf16 copy of z for
                the matmuls. Stats matmuls with a [128,128] ones/D stationary
                produce broadcast mean / E[z^2] tiles directly in PSUM.
                For the last LN, DMA the fp32 result out instead of mirroring."""
                for c in range(N):
                    t0, t1 = c * NC_CHUNK, (c + 1) * NC_CHUNK
                    bc_m = psp.tile([128, NC_CHUNK], F32, name=f"{pref}bm{b}_{c}", tag="ps")
                    for o in range(ND):
                        nc.tensor.matmul(bc_m[:], ones_sq[:], Z[o][:, t0:t1],
                                         start=(o == 0), stop=(o == ND - 1))
                    bc_q = psp.tile([128, NC_CHUNK], F32, name=f"{pref}bq{b}_{c}", tag="ps")
                    for o in range(ND):
                        zsq = scr.tile([128, NC_CHUNK], F32R, name=f"{pref}zs{b}_{c}_{o}", tag="zsq")
                        nc.scalar.activation(zsq[:], Z[o][:, t0:t1], AF.Square)
                        nc.tensor.matmul(bc_q[:], ones_sq[:], zsq[:],
                                         start=(o == 0), stop=(o == ND - 1))
                    # var = E[z^2] - mean^2, in place on the mean-square tile
                    msq = bcp.tile([128, NC_CHUNK], F32, name=f"{pref}ms{b}_{c}", tag="msq")
                    nc.scalar.activation(msq[:], bc_m[:], AF.Square)
                    nc.vector.tensor_tensor(msq[:], bc_q[:], msq[:], ALU.subtract)
                    rstd = bcp.tile([128, NC_CHUNK], F32R, name=f"{pref}rs{b}_{c}", tag="rstd")
                    nc.scalar.activation(rstd[:], msq[:], AF.Abs_reciprocal_sqrt,
                                         bias=eps_col[:])
                    for o in range(ND):
                        zs = Z[o][:, t0:t1]
                        nc.vector.tensor_tensor(zs, zs, bc_m[:], ALU.subtract)
                        if not ln_affine:
                            nc.vector.tensor_tensor(zs, zs, rstd[:], ALU.mult)
                        else:
                            nc.vector.scalar_tensor_tensor(
                                zs, zs, scol(pref + "_s", o), rstd[:],
                                ALU.mult, ALU.mult)
                            if ln_bias:
                                nc.vector.tensor_scalar(
                                    zs, zs, scol(pref + "_b", o), None, ALU.add)
                        if final:
                            nc.sync.dma_start(
                                out_ap[o * 128:(o + 1) * 128, t0:t1],
                                zs.bitcast(F32))
                        else:
                            nc.vector.tensor_copy(X[o][:, t0:t1], zs)

            for b in range(n_blocks):
                sc_sb = scalp.tile([128, NSCAL], F32, name=f"sc{b}", tag="sc")
                nc.sync.dma_start(sc_sb[:], sc_ap[b])

                def scol(name, i):
                    return sc_sb[:, _COLS[name] + i:_COLS[name] + i + 1]

                # ---------------- Phase A: K, V -> local KV/sumK ----------
                kvs_loc = kvsp.tile([128, 2 * NO * N], F32, name=f"kvl{b}", tag="kvl")
                # col layout: kv[o][n] at o*8+n, sumk[o][n] at o*8+4+n
                for o in range(NO):
                    wkv_sb = wp.tile([128, 2 * D], BF16, name=f"wkv{b}_{o}", tag="wkv")
                    nc.gpsimd.dma_start(wkv_sb[:], wkv_ap[b, o])
                    for n in range(N):
                        t0, t1 = n * NC_CHUNK, (n + 1) * NC_CHUNK
                        ps_k = psp.tile([128, NC_CHUNK], F32, name=f"psk{b}_{o}_{n}", tag="ps")
                        for d in range(ND):
                            nc.tensor.matmul(
                                ps_k[:], wkv_sb[:, d * 128:(d + 1) * 128],
                                X[d][:, t0:t1], start=(d == 0), stop=(d == ND - 1))
                        ps_v = psp.tile([128, NC_CHUNK], F32, name=f"psv{b}_{o}_{n}", tag="ps")
                        for d in range(ND):
                            nc.tensor.matmul(
                                ps_v[:], wkv_sb[:, D + d * 128:D + (d + 1) * 128],
                                X[d][:, t0:t1], start=(d == 0), stop=(d == ND - 1))
                        # kf = exp(min(k+bk,0)) + relu(k+bk)
                        y1 = bscr.tile([128, NC_CHUNK], BF16, name=f"y1_{b}_{o}_{n}", tag="bscr")
                        nc.scalar.activation(y1[:], ps_k[:], AF.Relu,
                                             bias=scol("nbk", o), scale=-1.0)
                        t2 = bscr.tile([128, NC_CHUNK], BF16, name=f"t2_{b}_{o}_{n}", tag="bscr")
                        nc.scalar.activation(t2[:], ps_k[:], AF.Relu, bias=scol("bk", o))
                        nc.scalar.activation(y1[:], y1[:], AF.Exp, scale=-1.0)
                        vsb = scr.tile([128, NC_CHUNK], BF16, name=f"vs_{b}_{o}_{n}", tag="vsb")
                        nc.vector.tensor_copy(vsb[:], ps_v[:])
                        # fp32 out: accum_out precision follows the out dtype
                        kf = fscr.tile([128, NC_CHUNK], F32, name=f"kf_{b}_{o}_{n}", tag="fscr")
                        nc.vector.scalar_tensor_tensor(
                            kf[:], y1[:], 0.0, t2[:], ALU.add, ALU.add,
                            accum_out=kvs_loc[:, o * 8 + 4 + n:o * 8 + 5 + n])
                        nc.vector.scalar_tensor_tensor(
                            kf[:], kf[:], 1.0, vsb[:], ALU.mult, ALU.mult,
                            accum_out=kvs_loc[:, o * 8 + n:o * 8 + 1 + n])

                # ---------------- AllReduce of KV/sumK --------------------
                cc_in = dramp.tile([128, 2 * NO * N], F32, name=f"cci{b}", tag="cci")
                cc_out = dramp.tile([128, 2 * NO * N], F32, name=f"cco{b}", tag="cco")
                nc.sync.dma_start(cc_in[:], kvs_loc[:])
                nc.gpsimd.collective_compute(
                    "AllReduce", mybir.AluOpType.add,
                    replica_groups=[list(range(N_CORES))],
                    ins=[cc_in.opt()], outs=[cc_out.opt()])
                kvs = kvsp.tile([128, 2 * NO * N], F32, name=f"kvg{b}", tag="kvg")
                nc.sync.dma_start(kvs[:], cc_out[:])
                # fold V-bias into KV: kv += bv * sumk  (exact)
                for o in range(NO):
                    nc.vector.scalar_tensor_tensor(
                        kvs[:, o * 8:o * 8 + 4], kvs[:, o * 8 + 4:o * 8 + 8],
                        scol("bv", o), kvs[:, o * 8:o * 8 + 4], ALU.mult, ALU.add)
                if debug and b == 0:
                    nc.sync.dma_start(dbg_aps["d_kvs"][:], kvs[:])

                # ------------- Phase B, split in n-halves -----------------
                for nh in range(2):
                    ns = (2 * nh, 2 * nh + 1)
                    qf = {}
                    # B1: Q projections + feature map (independent of the CC)
                    for o in range(NO):
                        wq_sb = wp.tile([128, D], BF16, name=f"wq{b}_{o}_{nh}", tag="wq")
                        nc.scalar.dma_start(wq_sb[:], wq_ap[b, o])
                        for n in ns:
                            t0, t1 = n * NC_CHUNK, (n + 1) * NC_CHUNK
                            ps_q = psp.tile([128, NC_CHUNK], F32, name=f"psq{b}_{o}_{n}", tag="ps")
                            for d in range(ND):
                                nc.tensor.matmul(
                                    ps_q[:], wq_sb[:, d * 128:(d + 1) * 128],
                                    X[d][:, t0:t1], start=(d == 0), stop=(d == ND - 1))
                            y1q = bscr.tile([128, NC_CHUNK], BF16, name=f"yq_{b}_{o}_{n}", tag="bscr")
                            nc.scalar.activation(y1q[:], ps_q[:], AF.Relu,
                                                 bias=scol("nbq", o), scale=-1.0)
                            t2q = bscr.tile([128, NC_CHUNK], BF16, name=f"qt2_{b}_{o}_{n}", tag="bscr")
                            nc.scalar.activation(t2q[:], ps_q[:], AF.Relu, bias=scol("bq", o))
                            nc.scalar.activation(y1q[:], y1q[:], AF.Exp, scale=-1.0)
                            qt = qfp.tile([128, NC_CHUNK], BF16, name=f"qf{b}_{o}_{n}", tag=f"qf{o}")
                            nc.vector.tensor_tensor(qt[:], y1q[:], t2q[:], ALU.add)
                            qf[(o, n)] = qt

                    # B2: att = qf * kv / (qf*sumk + eps), in place on qf
                    for n in ns:
                        for o in range(NO):
                            qt = qf[(o, n)]
                            den = fscr.tile([128, NC_CHUNK], F32, name=f"dn_{b}_{o}_{n}", tag="fscr")
                            # on GpSimd: runs parallel to the DVE recip/att chain
                            nc.gpsimd.tensor_scalar(
                                den[:], qt[:], kvs[:, o * 8 + 4 + n:o * 8 + 5 + n],
                                ATT_EPS, ALU.mult, ALU.add)
                            nc.vector.reciprocal_approx_fast(den[:], den[:])
                            # recb = kv / den via the ACT per-partition scale
                            recb = bscr.tile([128, NC_CHUNK], BF16, name=f"rb_{b}_{o}_{n}", tag="bscr")
                            nc.scalar.activation(recb[:], den[:], AF.Copy,
                                                 scale=kvs[:, o * 8 + n:o * 8 + 1 + n])
                            nc.vector.tensor_tensor(qt[:], qt[:], recb[:], ALU.mult)
                            if debug and b == 0 and o == 0:
                                t0, t1 = n * NC_CHUNK, (n + 1) * NC_CHUNK
                                nc.sync.dma_start(dbg_aps["d_att0"][:, t0:t1], qt[:])

                    # B3: Wo -> gelu -> z = y + x
                    for o2 in range(ND):
                        wo_sb = wp.tile([128, HK], BF16, name=f"wo{b}_{o2}_{nh}", tag="wo")
                        nc.gpsimd.dma_start(wo_sb[:], wo_ap[b, o2])
                        for n in ns:
                            t0, t1 = n * NC_CHUNK, (n + 1) * NC_CHUNK
                            ps_y = psp.tile([128, NC_CHUNK], F32, name=f"psy{b}_{o2}_{n}", tag="ps")
                            for o in range(NO):
                                nc.tensor.matmul(
                                    ps_y[:], wo_sb[:, o * 128:(o + 1) * 128],
                                    qf[(o, n)][:], start=(o == 0), stop=(o == NO - 1))
                            yt = actp.tile([128, NC_CHUNK], F32R, name=f"y_{b}_{o2}_{n}", tag="actout")
                            nc.scalar.activation(yt[:], ps_y[:], AF.Gelu_apprx_tanh,
                                                 bias=scol("bo", o2))
                            nc.vector.tensor_tensor(Z[o2][:, t0:t1], yt[:], Z[o2][:, t0:t1],
                                                    ALU.add)

                if debug and b == 0:
                    nc.sync.dma_start(dbg_aps["d_z1"][:], Z[0][:].bitcast(F32))

                # ---------------- LN1: x = LN(z) --------------------------
                ln("ln1", b, scol)

                if debug and b == 0:
                    nc.sync.dma_start(dbg_aps["d_ln1"][:], X[0][:])

                # ------- FFN: z2 = gelu(gelu(x@W1+b1)@W2+b2) + x ----------
                for c in range(N):
                    t0, t1 = c * NC_CHUNK, (c + 1) * NC_CHUNK
                    hs = []
                    for f in range(NF):
                        w1t = w1p.tile([128, D], BF16, name=f"w1_{b}_{c}_{f}", tag="w1")
                        (nc.sync if f % 2 == 0 else nc.scalar).dma_start(
                            w1t[:], w1_ap[b, f])
                        ps_h = psp.tile([128, NC_CHUNK], F32, name=f"psh{b}_{c}_{f}", tag="ps")
                        for d in range(ND):
                            nc.tensor.matmul(
                                ps_h[:], w1t[:, d * 128:(d + 1) * 128],
                                X[d][:, t0:t1], start=(d == 0), stop=(d == ND - 1))
                        ht = hp.tile([128, NC_CHUNK], BF16, name=f"h_{b}_{c}_{f}", tag="h")
                        nc.scalar.activation(ht[:], ps_h[:], AF.Gelu_apprx_tanh,
                                             bias=scol("b1", f))
                        if debug and b == 0 and f == 0 and c == 0:
                            nc.sync.dma_start(dbg_aps["d_h0"][:], ht[:])
                        hs.append(ht)
                    for o4 in range(ND):
                        ps2 = psp.tile([128, NC_CHUNK], F32, name=f"ps2{b}_{c}_{o4}", tag="ps")
                        for quart in range(4):
                            w2t = wp.tile([128, F // 4], BF16,
                                          name=f"w2_{b}_{c}_{o4}_{quart}", tag="w2")
                            nc.gpsimd.dma_start(w2t[:], w2_ap[b, o4, quart])
                            for j in range(NF // 4):
                                f = quart * (NF // 4) + j
                                nc.tensor.matmul(
                                    ps2[:], w2t[:, j * 128:(j + 1) * 128], hs[f][:],
                                    start=(f == 0), stop=(f == NF - 1))
                        gt = actp.tile([128, NC_CHUNK], F32R, name=f"g_{b}_{c}_{o4}", tag="actout")
                        nc.scalar.activation(gt[:], ps2[:], AF.Gelu_apprx_tanh,
                                             bias=scol("b2", o4))
                        nc.vector.tensor_tensor(Z[o4][:, t0:t1], gt[:], Z[o4][:, t0:t1],
                                                ALU.add)

                if debug and b == 0:
                    nc.sync.dma_start(dbg_aps["d_ffnz"][:], Z[0][:].bitcast(F32))

                # ---------------- LN2: x = LN(z2) -------------------------
                ln("ln2", b, scol, final=(b == n_blocks - 1))

    nc.compile()
    return nc


def _prep_inputs(inputs, n_blocks):
    """Host-side: shard x over sequence, pre-transpose to feature-major
    bf16, pack weights as contiguous lhsT tiles, pack biases/scales."""
    import ml_dtypes

    bf16 = ml_dtypes.bfloat16
    x = np.asarray(inputs["x"], dtype=np.float32)
    Wq = np.asarray(inputs["Wq"], dtype=np.float32)
    Wk = np.asarray(inputs["Wk"], dtype=np.float32)
    Wv = np.asarray(inputs["Wv"], dtype=np.float32)
    Wo = np.asarray(inputs["Wo"], dtype=np.float32)
    W1 = np.asarray(inputs["W1"], dtype=np.float32)
    W2 = np.asarray(inputs["W2"], dtype=np.float32)

    def qkv_pack(arr):  # [B,H,D,K] -> [B,D,HK] -> [nb, o, p(d), dd, m(hk)]
        a2 = arr.transpose(0, 2, 1, 3).reshape(B, D, HK)[:n_blocks]
        a5 = a2.reshape(n_blocks, ND, 128, NO, 128)     # [b, dd, p(d), o, m(hk)]
        # lhsT tile[p(d in dd), dd*128+m(hk)] = W[b, d=dd*128+p, hk=o*128+m]
        return a5.transpose(0, 3, 2, 1, 4).astype(bf16)  # [b, o, p, dd, m]

    wq = np.ascontiguousarray(qkv_pack(Wq))
    wkv = np.ascontiguousarray(
        np.concatenate([qkv_pack(Wk).reshape(n_blocks, NO, 128, D),
                        qkv_pack(Wv).reshape(n_blocks, NO, 128, D)], axis=3))
    # Wo [B, HK, D]: tile[o2][p(hk in o), o*128+m(d in o2)] = Wo[b, o*128+p, o2*128+m]
    wo = np.ascontiguousarray(
        Wo[:n_blocks].reshape(n_blocks, NO, 128, ND, 128)
        .transpose(0, 3, 2, 1, 4).astype(bf16))          # [b, o2, p, o, m]
    # W1 [B, D, F]: tile[f][p(d in dd), dd*128+m(f)] = W1[b, d=dd*128+p, f=f*128+m]
    w1 = np.ascontiguousarray(
        W1[:n_blocks].reshape(n_blocks, ND, 128, NF, 128)
        .transpose(0, 3, 2, 1, 4).astype(bf16))          # [b, f, p, dd, m]
    # W2 [B, F, D]: tile[o4][p(f in ft), ft*128+m(d in o4)] = W2[b, ft*128+p, o4*128+m]
    w2 = np.ascontiguousarray(
        W2[:n_blocks].reshape(n_blocks, NF, 128, ND, 128)
        .transpose(0, 3, 2, 1, 4)                        # [b, o4, p, ft, m]
        .reshape(n_blocks, ND, 128, 4, (NF // 4) * 128)  # split f-quarters
        .transpose(0, 1, 3, 2, 4).astype(bf16))          # [b, o4, quart, p, cols]

    scal = np.zeros((n_blocks, 128, NSCAL), np.float32)

    def put(name, arr2d):  # arr2d [n_blocks, width] -> 128-chunk columns
        w = arr2d.shape[1]
        ncol = w // 128
        scal[:, :, _COLS[name]:_COLS[name] + ncol] = \
            arr2d.reshape(n_blocks, ncol, 128).transpose(0, 2, 1)

    bq2 = np.asarray(inputs["bq"], np.float32).reshape(B, HK)[:n_blocks]
    bk2 = np.asarray(inputs["bk"], np.float32).reshape(B, HK)[:n_blocks]
    put("bq", bq2)
    put("bk", bk2)
    put("nbq", -bq2)
    put("nbk", -bk2)
    put("bv", np.asarray(inputs["bv"], np.float32).reshape(B, HK)[:n_blocks])
    put("bo", np.asarray(inputs["bo"], np.float32)[:n_blocks])
    put("b1", np.asarray(inputs["b1"], np.float32)[:n_blocks])
    put("b2", np.asarray(inputs["b2"], np.float32)[:n_blocks])
    put("ln1_s", np.asarray(inputs["ln1_s"], np.float32)[:n_blocks])
    put("ln1_b", np.asarray(inputs["ln1_b"], np.float32)[:n_blocks])
    put("ln2_s", np.asarray(inputs["ln2_s"], np.float32)[:n_blocks])
    put("ln2_b", np.asarray(inputs["ln2_b"], np.float32)[:n_blocks])

    ln_bias = bool(
        np.any(np.asarray(inputs["ln1_b"])[:n_blocks]) or
        np.any(np.asarray(inputs["ln2_b"])[:n_blocks]))
    ln_affine = ln_bias or bool(
        np.any(np.asarray(inputs["ln1_s"])[:n_blocks] != 1.0) or
        np.any(np.asarray(inputs["ln2_s"])[:n_blocks] != 1.0))

    in_maps = []
    for core in range(N_CORES):
        s0 = core * SEQ_SH
        xc = np.ascontiguousarray(
            x[:, s0:s0 + SEQ_SH, :].transpose(2, 0, 1).reshape(D, TPC).astype(bf16))
        in_maps.append({
            "x_fm": xc, "wkv": wkv, "wq": wq, "wo": wo,
            "w1": w1, "w2": w2, "scal": scal,
        })
    return in_maps, ln_affine, ln_bias


def run(inputs, n_blocks=B, trace=False, debug=False):
    from concourse.bass_utils import run_bass_kernel_spmd

    in_maps, ln_affine, ln_bias = _prep_inputs(inputs, n_blocks)
    key = (n_blocks, ln_affine, ln_bias, debug)
    if key not in _BUILD_CACHE:
        _BUILD_CACHE[key] = _build(n_blocks, ln_affine=ln_affine,
                                   ln_bias=ln_bias, debug=debug)
    nc = _BUILD_CACHE[key]
    res = run_bass_kernel_spmd(nc, in_maps, list(range(N_CORES)), trace=trace)
    # gather: per-core [D, TPC] feature-major -> [N, S, D]
    out = np.empty((N, S, D), np.float32)
    for core in range(N_CORES):
        s0 = core * SEQ_SH
        oc = np.asarray(res.results[core]["out_fm"]).astype(np.float32)  # [D, TPC]
        out[:, s0:s0 + SEQ_SH, :] = \
            oc.reshape(D, N, SEQ_SH).transpose(1, 2, 0)
    return out, res


def kernel(**inputs):
    out, _ = run(inputs, n_blocks=B, trace=False)
    return out


# revision 24
# speedup vs baseline: 1.2059x; 1.0970x over previous
"""Trainium2 Bass kernel for an 8-block linear-attention transformer.

Contract: kernel(**inputs) takes full unsharded inputs (as in
reference.setup_inputs()) and returns the full [N, S, D] output.

Sharding: sequence-parallel over the 16384 tokens -> 2048 tokens/core on
8 NeuronCores. The only cross-token coupling is the per-sample KV/sumK
sums of the linear attention; each core computes partial sums over its
local tokens and a tiny [128, 64] fp32 AllReduce per block produces the
global sums. Everything else is purely token-parallel.

v3 design notes:
- Matmul inputs bf16 (weights + X + att + h); fp32r for the Z residual
  master, LN internals and gelu outputs feeding Z. PSUM/stats fp32.
- LayerNorm: stats matmuls use a full [128,128] ones/D stationary so the
  8-matmul accumulation lands the broadcast mean/E[z^2] tiles directly
  (no thin M=1 matmuls, no separate broadcast matmuls, no row chains);
  rstd via one wide Abs_reciprocal_sqrt; apply is two tensor_tensor ops
  with the subtraction done in place on the dying Z tile.
- FFN second matmul accumulates all 32 f-tiles in PSUM per chunk.
- scalar_tensor_tensor runs 1x on DVE regardless of dtype, so hot
  elementwise ops are tensor_tensor (2x bf16) where possible; the
  attention 1/den is folded into the PSUM->SBUF ACT copy via its
  per-partition scale operand.
- ACT table sets: exp_and_others -> gelu_apprx_tanh -> abs_r_sqrt,
  ~5 loads per block.
- All Q projections for a half of the token chunks are issued before
  anything depending on the AllReduce, hiding the collective.
"""

import sys

sys.path.insert(0, "/opt/trn_rl_repo")

import numpy as np

# dims (hardcoded; must match reference.py)
B, H, D, K, F = 8, 16, 1024, 64, 4096
N, S = 4, 4096
HK = H * K  # 1024
N_CORES = 8
TPC = (N * S) // N_CORES  # tokens per core = 2048
SEQ_SH = S // N_CORES     # 512 local tokens per sample
NC_CHUNK = TPC // N       # 512 = one sample's local tokens (= chunk)
ND = D // 128             # 8 d-tiles
NO = HK // 128            # 8 attention-feature tiles
NF = F // 128             # 32 ffn tiles
LN_EPS = 1e-6
ATT_EPS = 1e-6

# bias/scale column layout in the packed per-block fp32 scalar tensor
_COLS = {}
_c = 0
for _name, _n in [("bq", NO), ("bk", NO), ("bv", NO), ("nbq", NO), ("nbk", NO),
                  ("bo", ND), ("ln1_s", ND), ("ln1_b", ND),
                  ("ln2_s", ND), ("ln2_b", ND), ("b2", ND), ("b1", NF)]:
    _COLS[_name] = _c
    _c += _n
NSCAL = _c

_BUILD_CACHE = {}


def _build(n_blocks, ln_affine=False, ln_bias=False, debug=False):
    """Build the SPMD bass program (same program for all 8 cores)."""
    import concourse.bacc as bacc
    import concourse.tile as tile
    import concourse.mybir as mybir

    F32 = mybir.dt.float32
    F32R = mybir.dt.float32r
    BF16 = mybir.dt.bfloat16
    AF = mybir.ActivationFunctionType
    ALU = mybir.AluOpType

    nc = bacc.Bacc("TRN2", target_bir_lowering=False, debug=False,
                   num_devices=N_CORES)

    x_ap = nc.dram_tensor("x_fm", [D, TPC], BF16, kind="ExternalInput").ap()
    wkv_ap = nc.dram_tensor("wkv", [n_blocks, NO, 128, 2 * D], BF16, kind="ExternalInput").ap()
    wq_ap = nc.dram_tensor("wq", [n_blocks, NO, 128, D], BF16, kind="ExternalInput").ap()
    wo_ap = nc.dram_tensor("wo", [n_blocks, ND, 128, HK], BF16, kind="ExternalInput").ap()
    w1_ap = nc.dram_tensor("w1", [n_blocks, NF, 128, D], BF16, kind="ExternalInput").ap()
    # w2 packed per (o4, f-quarter): [n_blocks, ND, 4, 128, F/4]
    w2_ap = nc.dram_tensor("w2", [n_blocks, ND, 4, 128, F // 4], BF16, kind="ExternalInput").ap()
    sc_ap = nc.dram_tensor("scal", [n_blocks, 128, NSCAL], F32, kind="ExternalInput").ap()
    out_ap = nc.dram_tensor("out_fm", [D, TPC], F32, kind="ExternalOutput").ap()
    dbg_aps = {}
    if debug:
        for nm, shp, dt in [("d_kvs", [128, 64], F32), ("d_att0", [128, TPC], BF16),
                            ("d_z1", [128, TPC], F32), ("d_ln1", [128, TPC], BF16),
                            ("d_h0", [128, 512], BF16), ("d_ffnz", [128, TPC], F32)]:
            dbg_aps[nm] = nc.dram_tensor(nm, shp, dt, kind="ExternalOutput").ap()

    from contextlib import ExitStack

    with tile.TileContext(nc) as tc:
        with ExitStack() as _es:
            def _pool(*a, **k):
                return _es.enter_context(tc.tile_pool(*a, **k))

            resid = _pool(name="resid", bufs=1)
            consts = _pool(name="consts", bufs=1)
            scalp = _pool(name="scalp", bufs=1)
            kvsp = _pool(name="kvsp", bufs=2)
            wp = _pool(name="wp", bufs=2)
            w1p = _pool(name="w1p", bufs=4)
            w2p = _pool(name="w2p", bufs=3)
            hp = _pool(name="hp", bufs=32)
            qfp = _pool(name="qfp", bufs=3)
            bscr = _pool(name="bscr", bufs=3)
            fscr = _pool(name="fscr", bufs=2)
            scr = _pool(name="scr", bufs=2)
            actp = _pool(name="actp", bufs=2)
            bcp = _pool(name="bcp", bufs=2)
            psp = _pool(name="ps", bufs=8, space="PSUM")
            dramp = _pool(name="dram", bufs=2, space="DRAM")

            # persistent residual-stream buffers (feature-major)
            X = [resid.tile([128, TPC], BF16, name=f"X{d}", tag=f"X{d}")
                 for d in range(ND)]
            Z = [resid.tile([128, TPC], F32R, name=f"Z{d}", tag=f"Z{d}")
                 for d in range(ND)]

            # constants
            ones_f = consts.tile([128, 128], F32)
            nc.vector.memset(ones_f[:], 1.0 / D)
            ones_sq = consts.tile([128, 128], F32R)  # 1/D stats+broadcast lhsT
            nc.scalar.activation(ones_sq[:], ones_f[:], AF.Copy)
            eps_col = consts.tile([128, 1], F32)
            nc.vector.memset(eps_col[:], LN_EPS)

            # load input activations; Z is the fp32 residual-stream master,
            # X the bf16 matmul mirror
            for d in range(ND):
                nc.gpsimd.dma_start(X[d][:], x_ap[d * 128:(d + 1) * 128, :])
                nc.scalar.activation(Z[d][:], X[d][:], AF.Copy)

            def ln(pref, b, scol, final=False):
                """z = LN(z) in place (fp32 master); x = bf16 copy of z for
                the matmuls. Stats matmuls with a [128,128] ones/D stationary
                produce broadcast mean / E[z^2] tiles directly in PSUM.
                For the last LN, DMA the fp32 result out instead of mirroring."""
                for c in range(N):
                    t0, t1 = c * NC_CHUNK, (c + 1) * NC_CHUNK
                    bc_m = psp.tile([128, NC_CHUNK], F32, name=f"{pref}bm{b}_{c}", tag="ps")
                    for o in range(ND):
                        nc.tensor.matmul(bc_m[:], ones_sq[:], Z[o][:, t0:t1],
                                         start=(o == 0), stop=(o == ND - 1))
                    bc_q = psp.tile([128, NC_CHUNK], F32, name=f"{pref}bq{b}_{c}", tag="ps")
                    for o in range(ND):
                        zsq = scr.tile([128, NC_CHUNK], F32R, name=f"{pref}zs{b}_{c}_{o}", tag="zsq")
                        nc.scalar.activation(zsq[:], Z[o][:, t0:t1], AF.Square)
                        nc.tensor.matmul(bc_q[:], ones_sq[:], zsq[:],
                                         start=(o == 0), stop=(o == ND - 1))
                    # var = E[z^2] - mean^2, in place on the mean-square tile
                    msq = bcp.tile([128, NC_CHUNK], F32, name=f"{pref}ms{b}_{c}", tag="msq")
                    nc.scalar.activation(msq[:], bc_m[:], AF.Square)
                    nc.vector.tensor_tensor(msq[:], bc_q[:], msq[:], ALU.subtract)
                    rstd = bcp.tile([128, NC_CHUNK], F32R, name=f"{pref}rs{b}_{c}", tag="rstd")
                    nc.scalar.activation(rstd[:], msq[:], AF.Abs_reciprocal_sqrt,
                                         bias=eps_col[:])
                    for o in range(ND):
                        zs = Z[o][:, t0:t1]
                        nc.vector.tensor_tensor(zs, zs, bc_m[:], ALU.subtract)
                        if not ln_affine:
                            nc.vector.tensor_tensor(zs, zs, rstd[:], ALU.mult)
                        else:
                            nc.vector.scalar_tensor_tensor(
                                zs, zs, scol(pref + "_s", o), rstd[:],
                                ALU.mult, ALU.mult)
                            if ln_bias:
                                nc.vector.tensor_scalar(
                                    zs, zs, scol(pref + "_b", o), None, ALU.add)
                        if final:
                            nc.sync.dma_start(
                                out_ap[o * 128:(o + 1) * 128, t0:t1],
                                zs.bitcast(F32))
                        else:
                            nc.vector.tensor_copy(X[o][:, t0:t1], zs)

            for b in range(n_blocks):
                sc_sb = scalp.tile([128, NSCAL], F32, name=f"sc{b}", tag="sc")
                nc.sync.dma_start(sc_sb[:], sc_ap[b])

                def scol(name, i):
                    return sc_sb[:, _COLS[name] + i:_COLS[name] + i + 1]

                # ---------------- Phase A: K, V -> local KV/sumK ----------
                kvs_loc = kvsp.tile([128, 2 * NO * N], F32, name=f"kvl{b}", tag="kvl")
                # col layout: kv[o][n] at o*8+n, sumk[o][n] at o*8+4+n
                for o in range(NO):
                    wkv_sb = wp.tile([128, 2 * D], BF16, name=f"wkv{b}_{o}", tag="wkv")
                    nc.gpsimd.dma_start(wkv_sb[:], wkv_ap[b, o])
                    for n in range(N):
                        t0, t1 = n * NC_CHUNK, (n + 1) * NC_CHUNK
                        ps_k = psp.tile([128, NC_CHUNK], F32, name=f"psk{b}_{o}_{n}", tag="ps")
                        for d in range(ND):
                            nc.tensor.matmul(
                                ps_k[:], wkv_sb[:, d * 128:(d + 1) * 128],
                                X[d][:, t0:t1], start=(d == 0), stop=(d == ND - 1))
                        ps_v = psp.tile([128, NC_CHUNK], F32, name=f"psv{b}_{o}_{n}", tag="ps")
                        for d in range(ND):
                            nc.tensor.matmul(
                                ps_v[:], wkv_sb[:, D + d * 128:D + (d + 1) * 128],
                                X[d][:, t0:t1], start=(d == 0), stop=(d == ND - 1))
                        # kf = exp(min(k+bk,0)) + relu(k+bk)
                        y1 = bscr.tile([128, NC_CHUNK], BF16, name=f"y1_{b}_{o}_{n}", tag="bscr")
                        nc.scalar.activation(y1[:], ps_k[:], AF.Relu,
                                             bias=scol("nbk", o), scale=-1.0)
                        t2 = bscr.tile([128, NC_CHUNK], BF16, name=f"t2_{b}_{o}_{n}", tag="bscr")
                        nc.scalar.activation(t2[:], ps_k[:], AF.Relu, bias=scol("bk", o))
                        nc.scalar.activation(y1[:], y1[:], AF.Exp, scale=-1.0)
                        vsb = scr.tile([128, NC_CHUNK], BF16, name=f"vs_{b}_{o}_{n}", tag="vsb")
                        nc.vector.tensor_copy(vsb[:], ps_v[:])
                        # fp32 out: accum_out precision follows the out dtype
                        kf = fscr.tile([128, NC_CHUNK], F32, name=f"kf_{b}_{o}_{n}", tag="fscr")
                        nc.vector.scalar_tensor_tensor(
                            kf[:], y1[:], 0.0, t2[:], ALU.add, ALU.add,
                            accum_out=kvs_loc[:, o * 8 + 4 + n:o * 8 + 5 + n])
                        nc.vector.scalar_tensor_tensor(
                            kf[:], kf[:], 1.0, vsb[:], ALU.mult, ALU.mult,
                            accum_out=kvs_loc[:, o * 8 + n:o * 8 + 1 + n])

                # ---------------- AllReduce of KV/sumK --------------------
                cc_in = dramp.tile([128, 2 * NO * N], F32, name=f"cci{b}", tag="cci")
                cc_out = dramp.tile([128, 2 * NO * N], F32, name=f"cco{b}", tag="cco")
                nc.sync.dma_start(cc_in[:], kvs_loc[:])
                nc.gpsimd.collective_compute(
                    "AllReduce", mybir.AluOpType.add,
                    replica_groups=[list(range(N_CORES))],
                    ins=[cc_in.opt()], outs=[cc_out.opt()])
                kvs = kvsp.tile([128, 2 * NO * N], F32, name=f"kvg{b}", tag="kvg")
                nc.sync.dma_start(kvs[:], cc_out[:])
                # fold V-bias into KV: kv += bv * sumk  (exact)
                for o in range(NO):
                    nc.vector.scalar_tensor_tensor(
                        kvs[:, o * 8:o * 8 + 4], kvs[:, o * 8 + 4:o * 8 + 8],
                        scol("bv", o), kvs[:, o * 8:o * 8 + 4], ALU.mult, ALU.add)
                if debug and b == 0:
                    nc.sync.dma_start(dbg_aps["d_kvs"][:], kvs[:])

                # ------------- Phase B, split in n-halves -----------------
                for nh in range(2):
                    ns = (2 * nh, 2 * nh + 1)
                    qf = {}
                    # B1: Q projections + feature map (independent of the CC)
                    for o in range(NO):
                        wq_sb = wp.tile([128, D], BF16, name=f"wq{b}_{o}_{nh}", tag="wq")
                        nc.scalar.dma_start(wq_sb[:], wq_ap[b, o])
                        for n in ns:
                            t0, t1 = n * NC_CHUNK, (n + 1) * NC_CHUNK
                            ps_q = psp.tile([128, NC_CHUNK], F32, name=f"psq{b}_{o}_{n}", tag="ps")
                            for d in range(ND):
                                nc.tensor.matmul(
                                    ps_q[:], wq_sb[:, d * 128:(d + 1) * 128],
                                    X[d][:, t0:t1], start=(d == 0), stop=(d == ND - 1))
                            y1q = bscr.tile([128, NC_CHUNK], BF16, name=f"yq_{b}_{o}_{n}", tag="bscr")
                            nc.scalar.activation(y1q[:], ps_q[:], AF.Relu,
                                                 bias=scol("nbq", o), scale=-1.0)
                            t2q = bscr.tile([128, NC_CHUNK], BF16, name=f"qt2_{b}_{o}_{n}", tag="bscr")
                            nc.scalar.activation(t2q[:], ps_q[:], AF.Relu, bias=scol("bq", o))
                            nc.scalar.activation(y1q[:], y1q[:], AF.Exp, scale=-1.0)
                            qt = qfp.tile([128, NC_CHUNK], BF16, name=f"qf{b}_{o}_{n}", tag=f"qf{o}")
                            nc.vector.tensor_tensor(qt[:], y1q[:], t2q[:], ALU.add)
                            qf[(o, n)] = qt

                    # B2: att = qf * kv / (qf*sumk + eps), in place on qf
                    for n in ns:
                        for o in range(NO):
                            qt = qf[(o, n)]
                            den = fscr.tile([128, NC_CHUNK], F32, name=f"dn_{b}_{o}_{n}", tag="fscr")
                            # on GpSimd: runs parallel to the DVE recip/att chain
                            nc.gpsimd.tensor_scalar(
                                den[:], qt[:], kvs[:, o * 8 + 4 + n:o * 8 + 5 + n],
                                ATT_EPS, ALU.mult, ALU.add)
                            nc.vector.reciprocal_approx_fast(den[:], den[:])
                            # recb = kv / den via the ACT per-partition scale
                            recb = bscr.tile([128, NC_CHUNK], BF16, name=f"rb_{b}_{o}_{n}", tag="bscr")
                            nc.scalar.activation(recb[:], den[:], AF.Copy,
                                                 scale=kvs[:, o * 8 + n:o * 8 + 1 + n])
                            nc.vector.tensor_tensor(qt[:], qt[:], recb[:], ALU.mult)
                            if debug and b == 0 and o == 0:
                                t0, t1 = n * NC_CHUNK, (n + 1) * NC_CHUNK
                                nc.sync.dma_start(dbg_aps["d_att0"][:, t0:t1], qt[:])

                    # B3: Wo -> gelu -> z = y + x
                    for o2 in range(ND):
                        wo_sb = wp.tile([128, HK], BF16, name=f"wo{b}_{o2}_{nh}", tag="wo")
                        nc.sync.dma_start(wo_sb[:], wo_ap[b, o2])
                        for n in ns:
                            t0, t1 = n * NC_CHUNK, (n + 1) * NC_CHUNK
                            ps_y = psp.tile([128, NC_CHUNK], F32, name=f"psy{b}_{o2}_{n}", tag="ps")
                            for o in range(NO):
                                nc.tensor.matmul(
                                    ps_y[:], wo_sb[:, o * 128:(o + 1) * 128],
                                    qf[(o, n)][:], start=(o == 0), stop=(o == NO - 1))
                            yt = actp.tile([128, NC_CHUNK], BF16, name=f"y_{b}_{o2}_{n}", tag="actout")
                            nc.scalar.activation(yt[:], ps_y[:], AF.Gelu_apprx_tanh,
                                                 bias=scol("bo", o2))
                            nc.vector.tensor_tensor(Z[o2][:, t0:t1], yt[:], Z[o2][:, t0:t1],
                                                    ALU.add)

                if debug and b == 0:
                    nc.sync.dma_start(dbg_aps["d_z1"][:], Z[0][:].bitcast(F32))

                # ---------------- LN1: x = LN(z) --------------------------
                ln("ln1", b, scol)

                if debug and b == 0:
                    nc.sync.dma_start(dbg_aps["d_ln1"][:], X[0][:])

                # ------- FFN: z2 = gelu(gelu(x@W1+b1)@W2+b2) + x ----------
                for c in range(N):
                    t0, t1 = c * NC_CHUNK, (c + 1) * NC_CHUNK
                    hs = []
                    for f in range(NF):
                        w1t = w1p.tile([128, D], BF16, name=f"w1_{b}_{c}_{f}", tag="w1")
                        (nc.sync if f % 2 == 0 else nc.scalar).dma_start(
                            w1t[:], w1_ap[b, f])
                        ps_h = psp.tile([128, NC_CHUNK], F32, name=f"psh{b}_{c}_{f}", tag="ps")
                        for d in range(ND):
                            nc.tensor.matmul(
                                ps_h[:], w1t[:, d * 128:(d + 1) * 128],
                                X[d][:, t0:t1], start=(d == 0), stop=(d == ND - 1))
                        ht = hp.tile([128, NC_CHUNK], BF16, name=f"h_{b}_{c}_{f}", tag="h")
                        nc.scalar.activation(ht[:], ps_h[:], AF.Gelu_apprx_tanh,
                                             bias=scol("b1", f))
                        if debug and b == 0 and f == 0 and c == 0:
                            nc.sync.dma_start(dbg_aps["d_h0"][:], ht[:])
                        hs.append(ht)
                    for o4 in range(ND):
                        ps2 = psp.tile([128, NC_CHUNK], F32, name=f"ps2{b}_{c}_{o4}", tag="ps")
                        for quart in range(4):
                            w2t = w2p.tile([128, F // 4], BF16,
                                          name=f"w2_{b}_{c}_{o4}_{quart}", tag="w2")
                            nc.gpsimd.dma_start(w2t[:], w2_ap[b, o4, quart])
                            for j in range(NF // 4):
                                f = quart * (NF // 4) + j
                                nc.tensor.matmul(
                                    ps2[:], w2t[:, j * 128:(j + 1) * 128], hs[f][:],
                                    start=(f == 0), stop=(f == NF - 1))
                        gt = actp.tile([128, NC_CHUNK], BF16, name=f"g_{b}_{c}_{o4}", tag="actout")
                        nc.scalar.activation(gt[:], ps2[:], AF.Gelu_apprx_tanh,
                                             bias=scol("b2", o4))
                        nc.vector.tensor_tensor(Z[o4][:, t0:t1], gt[:], Z[o4][:, t0:t1],
                                                ALU.add)

                if debug and b == 0:
                    nc.sync.dma_start(dbg_aps["d_ffnz"][:], Z[0][:].bitcast(F32))

                # ---------------- LN2: x = LN(z2) -------------------------
                ln("ln2", b, scol, final=(b == n_blocks - 1))

    nc.compile()
    return nc


def _prep_inputs(inputs, n_blocks):
    """Host-side: shard x over sequence, pre-transpose to feature-major
    bf16, pack weights as contiguous lhsT tiles, pack biases/scales."""
    import ml_dtypes

    bf16 = ml_dtypes.bfloat16
    x = np.asarray(inputs["x"], dtype=np.float32)
    Wq = np.asarray(inputs["Wq"], dtype=np.float32)
    Wk = np.asarray(inputs["Wk"], dtype=np.float32)
    Wv = np.asarray(inputs["Wv"], dtype=np.float32)
    Wo = np.asarray(inputs["Wo"], dtype=np.float32)
    W1 = np.asarray(inputs["W1"], dtype=np.float32)
    W2 = np.asarray(inputs["W2"], dtype=np.float32)

    def qkv_pack(arr):  # [B,H,D,K] -> [B,D,HK] -> [nb, o, p(d), dd, m(hk)]
        a2 = arr.transpose(0, 2, 1, 3).reshape(B, D, HK)[:n_blocks]
        a5 = a2.reshape(n_blocks, ND, 128, NO, 128)     # [b, dd, p(d), o, m(hk)]
        # lhsT tile[p(d in dd), dd*128+m(hk)] = W[b, d=dd*128+p, hk=o*128+m]
        return a5.transpose(0, 3, 2, 1, 4).astype(bf16)  # [b, o, p, dd, m]

    wq = np.ascontiguousarray(qkv_pack(Wq))
    wkv = np.ascontiguousarray(
        np.concatenate([qkv_pack(Wk).reshape(n_blocks, NO, 128, D),
                        qkv_pack(Wv).reshape(n_blocks, NO, 128, D)], axis=3))
    # Wo [B, HK, D]: tile[o2][p(hk in o), o*128+m(d in o2)] = Wo[b, o*128+p, o2*128+m]
    wo = np.ascontiguousarray(
        Wo[:n_blocks].reshape(n_blocks, NO, 128, ND, 128)
        .transpose(0, 3, 2, 1, 4).astype(bf16))          # [b, o2, p, o, m]
    # W1 [B, D, F]: tile[f][p(d in dd), dd*128+m(f)] = W1[b, d=dd*128+p, f=f*128+m]
    w1 = np.ascontiguousarray(
        W1[:n_blocks].reshape(n_blocks, ND, 128, NF, 128)
        .transpose(0, 3, 2, 1, 4).astype(bf16))          # [b, f, p, dd, m]
    # W2 [B, F, D]: tile[o4][p(f in ft), ft*128+m(d in o4)] = W2[b, ft*128+p, o4*128+m]
    w2 = np.ascontiguousarray(
        W2[:n_blocks].reshape(n_blocks, NF, 128, ND, 128)
        .transpose(0, 3, 2, 1, 4)                        # [b, o4, p, ft, m]
        .reshape(n_blocks, ND, 128, 4, (NF // 4) * 128)  # split f-quarters
        .transpose(0, 1, 3, 2, 4).astype(bf16))          # [b, o4, quart, p, cols]

    scal = np.zeros((n_blocks, 128, NSCAL), np.float32)

    def put(name, arr2d):  # arr2d [n_blocks, width] -> 128-chunk columns
        w = arr2d.shape[1]
        ncol = w // 128
        scal[:, :, _COLS[name]:_COLS[name] + ncol] = \
            arr2d.reshape(n_blocks, ncol, 128).transpose(0, 2, 1)

    bq2 = np.asarray(inputs["bq"], np.float32).reshape(B, HK)[:n_blocks]
    bk2 = np.asarray(inputs["bk"], np.float32).reshape(B, HK)[:n_blocks]
    put("bq", bq2)
    put("bk", bk2)
    put("nbq", -bq2)
    put("nbk", -bk2)
    put("bv", np.asarray(inputs["bv"], np.float32).reshape(B, HK)[:n_blocks])
    put("bo", np.asarray(inputs["bo"], np.float32)[:n_blocks])
    put("b1", np.asarray(inputs["b1"], np.float32)[:n_blocks])
    put("b2", np.asarray(inputs["b2"], np.float32)[:n_blocks])
    put("ln1_s", np.asarray(inputs["ln1_s"], np.float32)[:n_blocks])
    put("ln1_b", np.asarray(inputs["ln1_b"], np.float32)[:n_blocks])
    put("ln2_s", np.asarray(inputs["ln2_s"], np.float32)[:n_blocks])
    put("ln2_b", np.asarray(inputs["ln2_b"], np.float32)[:n_blocks])

    ln_bias = bool(
        np.any(np.asarray(inputs["ln1_b"])[:n_blocks]) or
        np.any(np.asarray(inputs["ln2_b"])[:n_blocks]))
    ln_affine = ln_bias or bool(
        np.any(np.asarray(inputs["ln1_s"])[:n_blocks] != 1.0) or
        np.any(np.asarray(inputs["ln2_s"])[:n_blocks] != 1.0))

    in_maps = []
    for core in range(N_CORES):
        s0 = core * SEQ_SH
        xc = np.ascontiguousarray(
            x[:, s0:s0 + SEQ_SH, :].transpose(2, 0, 1).reshape(D, TPC).astype(bf16))
        in_maps.append({
            "x_fm": xc, "wkv": wkv, "wq": wq, "wo": wo,
            "w1": w1, "w2": w2, "scal": scal,
        })
    return in_maps, ln_affine, ln_bias


def run(inputs, n_blocks=B, trace=False, debug=False):
    from concourse.bass_utils import run_bass_kernel_spmd

    in_maps, ln_affine, ln_bias = _prep_inputs(inputs, n_blocks)
    key = (n_blocks, ln_affine, ln_bias, debug)
    if key not in _BUILD_CACHE:
        _BUILD_CACHE[key] = _build(n_blocks, ln_affine=ln_affine,
                                   ln_bias=ln_bias, debug=debug)
    nc = _BUILD_CACHE[key]
    res = run_bass_kernel_spmd(nc, in_maps, list(range(N_CORES)), trace=trace)
    # gather: per-core [D, TPC] feature-major -> [N, S, D]
    out = np.empty((N, S, D), np.float32)
    for core in range(N_CORES):
        s0 = core * SEQ_SH
        oc = np.asarray(res.results[core]["out_fm"]).astype(np.float32)  # [D, TPC]
        out[:, s0:s0 + SEQ_SH, :] = \
            oc.reshape(D, N, SEQ_SH).transpose(1, 2, 0)
    return out, res


def kernel(**inputs):
    out, _ = run(inputs, n_blocks=B, trace=False)
    return out


# revision 26
# speedup vs baseline: 1.2170x; 1.0092x over previous
"""Trainium2 Bass kernel for an 8-block linear-attention transformer.

Contract: kernel(**inputs) takes full unsharded inputs (as in
reference.setup_inputs()) and returns the full [N, S, D] output.

Sharding: sequence-parallel over the 16384 tokens -> 2048 tokens/core on
8 NeuronCores. The only cross-token coupling is the per-sample KV/sumK
sums of the linear attention; each core computes partial sums over its
local tokens and a tiny [128, 64] fp32 AllReduce per block produces the
global sums. Everything else is purely token-parallel.

v3 design notes:
- Matmul inputs bf16 (weights + X + att + h); fp32r for the Z residual
  master, LN internals and gelu outputs feeding Z. PSUM/stats fp32.
- LayerNorm: stats matmuls use a full [128,128] ones/D stationary so the
  8-matmul accumulation lands the broadcast mean/E[z^2] tiles directly
  (no thin M=1 matmuls, no separate broadcast matmuls, no row chains);
  rstd via one wide Abs_reciprocal_sqrt; apply is two tensor_tensor ops
  with the subtraction done in place on the dying Z tile.
- FFN second matmul accumulates all 32 f-tiles in PSUM per chunk.
- scalar_tensor_tensor runs 1x on DVE regardless of dtype, so hot
  elementwise ops are tensor_tensor (2x bf16) where possible; the
  attention 1/den is folded into the PSUM->SBUF ACT copy via its
  per-partition scale operand.
- ACT table sets: exp_and_others -> gelu_apprx_tanh -> abs_r_sqrt,
  ~5 loads per block.
- All Q projections for a half of the token chunks are issued before
  anything depending on the AllReduce, hiding the collective.
"""

import sys

sys.path.insert(0, "/opt/trn_rl_repo")

import numpy as np

# dims (hardcoded; must match reference.py)
B, H, D, K, F = 8, 16, 1024, 64, 4096
N, S = 4, 4096
HK = H * K  # 1024
N_CORES = 8
TPC = (N * S) // N_CORES  # tokens per core = 2048
SEQ_SH = S // N_CORES     # 512 local tokens per sample
NC_CHUNK = TPC // N       # 512 = one sample's local tokens (= chunk)
ND = D // 128             # 8 d-tiles
NO = HK // 128            # 8 attention-feature tiles
NF = F // 128             # 32 ffn tiles
LN_EPS = 1e-6
ATT_EPS = 1e-6

# bias/scale column layout in the packed per-block fp32 scalar tensor
_COLS = {}
_c = 0
for _name, _n in [("bq", NO), ("bk", NO), ("bv", NO), ("nbq", NO), ("nbk", NO),
                  ("bo", ND), ("ln1_s", ND), ("ln1_b", ND),
                  ("ln2_s", ND), ("ln2_b", ND), ("b2", ND), ("b1", NF)]:
    _COLS[_name] = _c
    _c += _n
NSCAL = _c

_BUILD_CACHE = {}


def _build(n_blocks, ln_affine=False, ln_bias=False, debug=False):
    """Build the SPMD bass program (same program for all 8 cores)."""
    import concourse.bacc as bacc
    import concourse.tile as tile
    import concourse.mybir as mybir

    F32 = mybir.dt.float32
    F32R = mybir.dt.float32r
    BF16 = mybir.dt.bfloat16
    AF = mybir.ActivationFunctionType
    ALU = mybir.AluOpType

    nc = bacc.Bacc("TRN2", target_bir_lowering=False, debug=False,
                   num_devices=N_CORES)

    x_ap = nc.dram_tensor("x_fm", [D, TPC], BF16, kind="ExternalInput").ap()
    wkv_ap = nc.dram_tensor("wkv", [n_blocks, NO, 128, 2 * D], BF16, kind="ExternalInput").ap()
    wq_ap = nc.dram_tensor("wq", [n_blocks, NO, 128, D], BF16, kind="ExternalInput").ap()
    wo_ap = nc.dram_tensor("wo", [n_blocks, ND, 128, HK], BF16, kind="ExternalInput").ap()
    w1_ap = nc.dram_tensor("w1", [n_blocks, NF, 128, D], BF16, kind="ExternalInput").ap()
    # w2 packed per (o4, f-quarter): [n_blocks, ND, 4, 128, F/4]
    w2_ap = nc.dram_tensor("w2", [n_blocks, ND, 4, 128, F // 4], BF16, kind="ExternalInput").ap()
    sc_ap = nc.dram_tensor("scal", [n_blocks, 128, NSCAL], F32, kind="ExternalInput").ap()
    out_ap = nc.dram_tensor("out_fm", [D, TPC], F32, kind="ExternalOutput").ap()
    dbg_aps = {}
    if debug:
        for nm, shp, dt in [("d_kvs", [128, 64], F32), ("d_att0", [128, TPC], BF16),
                            ("d_z1", [128, TPC], F32), ("d_ln1", [128, TPC], BF16),
                            ("d_h0", [128, 512], BF16), ("d_ffnz", [128, TPC], F32)]:
            dbg_aps[nm] = nc.dram_tensor(nm, shp, dt, kind="ExternalOutput").ap()

    from contextlib import ExitStack

    with tile.TileContext(nc) as tc:
        with ExitStack() as _es:
            def _pool(*a, **k):
                return _es.enter_context(tc.tile_pool(*a, **k))

            resid = _pool(name="resid", bufs=1)
            consts = _pool(name="consts", bufs=1)
            scalp = _pool(name="scalp", bufs=1)
            kvsp = _pool(name="kvsp", bufs=1)
            wp = _pool(name="wp", bufs=2)
            w1p = _pool(name="w1p", bufs=4)
            w2p = _pool(name="w2p", bufs=3)
            wop = _pool(name="wop", bufs=3)
            hp = _pool(name="hp", bufs=32)
            qfp = _pool(name="qfp", bufs=3)
            bscr = _pool(name="bscr", bufs=3)
            fscr = _pool(name="fscr", bufs=2)
            scr = _pool(name="scr", bufs=2)
            actp = _pool(name="actp", bufs=2)
            bcp = _pool(name="bcp", bufs=2)
            msqp = _pool(name="msqp", bufs=1)
            psp = _pool(name="ps", bufs=8, space="PSUM")
            dramp = _pool(name="dram", bufs=2, space="DRAM")

            # persistent residual-stream buffers (feature-major)
            X = [resid.tile([128, TPC], BF16, name=f"X{d}", tag=f"X{d}")
                 for d in range(ND)]
            Z = [resid.tile([128, TPC], F32R, name=f"Z{d}", tag=f"Z{d}")
                 for d in range(ND)]

            # constants
            ones_f = consts.tile([128, 128], F32)
            nc.vector.memset(ones_f[:], 1.0 / D)
            ones_sq = consts.tile([128, 128], F32R)  # 1/D stats+broadcast lhsT
            nc.scalar.activation(ones_sq[:], ones_f[:], AF.Copy)
            eps_col = consts.tile([128, 1], F32)
            nc.vector.memset(eps_col[:], LN_EPS)

            # load input activations; Z is the fp32 residual-stream master,
            # X the bf16 matmul mirror
            for d in range(ND):
                nc.gpsimd.dma_start(X[d][:], x_ap[d * 128:(d + 1) * 128, :])
                nc.scalar.activation(Z[d][:], X[d][:], AF.Copy)

            def ln(pref, b, scol, final=False):
                """z = LN(z) in place (fp32 master); x = bf16 copy of z for
                the matmuls. Stats matmuls with a [128,128] ones/D stationary
                produce broadcast mean / E[z^2] tiles directly in PSUM.
                For the last LN, DMA the fp32 result out instead of mirroring."""
                for c in range(N):
                    t0, t1 = c * NC_CHUNK, (c + 1) * NC_CHUNK
                    bc_m = psp.tile([128, NC_CHUNK], F32, name=f"{pref}bm{b}_{c}", tag="ps")
                    for o in range(ND):
                        nc.tensor.matmul(bc_m[:], ones_sq[:], Z[o][:, t0:t1],
                                         start=(o == 0), stop=(o == ND - 1))
                    bc_q = psp.tile([128, NC_CHUNK], F32, name=f"{pref}bq{b}_{c}", tag="ps")
                    for o in range(ND):
                        zsq = scr.tile([128, NC_CHUNK], F32R, name=f"{pref}zs{b}_{c}_{o}", tag="zsq")
                        nc.scalar.activation(zsq[:], Z[o][:, t0:t1], AF.Square)
                        nc.tensor.matmul(bc_q[:], ones_sq[:], zsq[:],
                                         start=(o == 0), stop=(o == ND - 1))
                    # var = E[z^2] - mean^2, in place on the mean-square tile
                    msq = msqp.tile([128, NC_CHUNK], F32, name=f"{pref}ms{b}_{c}", tag="msq")
                    nc.scalar.activation(msq[:], bc_m[:], AF.Square)
                    nc.vector.tensor_tensor(msq[:], bc_q[:], msq[:], ALU.subtract)
                    rstd = bcp.tile([128, NC_CHUNK], F32R, name=f"{pref}rs{b}_{c}", tag="rstd")
                    nc.scalar.activation(rstd[:], msq[:], AF.Abs_reciprocal_sqrt,
                                         bias=eps_col[:])
                    for o in range(ND):
                        zs = Z[o][:, t0:t1]
                        nc.vector.tensor_tensor(zs, zs, bc_m[:], ALU.subtract)
                        if not ln_affine:
                            nc.vector.tensor_tensor(zs, zs, rstd[:], ALU.mult)
                        else:
                            nc.vector.scalar_tensor_tensor(
                                zs, zs, scol(pref + "_s", o), rstd[:],
                                ALU.mult, ALU.mult)
                            if ln_bias:
                                nc.vector.tensor_scalar(
                                    zs, zs, scol(pref + "_b", o), None, ALU.add)
                        if final:
                            nc.sync.dma_start(
                                out_ap[o * 128:(o + 1) * 128, t0:t1],
                                zs.bitcast(F32))
                        else:
                            nc.vector.tensor_copy(X[o][:, t0:t1], zs)

            for b in range(n_blocks):
                sc_sb = scalp.tile([128, NSCAL], F32, name=f"sc{b}", tag="sc")
                nc.sync.dma_start(sc_sb[:], sc_ap[b])

                def scol(name, i):
                    return sc_sb[:, _COLS[name] + i:_COLS[name] + i + 1]

                # ---------------- Phase A: K, V -> local KV/sumK ----------
                kvs_loc = kvsp.tile([128, 2 * NO * N], F32, name=f"kvl{b}", tag="kvl")
                # col layout: kv[o][n] at o*8+n, sumk[o][n] at o*8+4+n
                for o in range(NO):
                    wkv_sb = wp.tile([128, 2 * D], BF16, name=f"wkv{b}_{o}", tag="wkv")
                    nc.sync.dma_start(wkv_sb[:], wkv_ap[b, o])
                    for n in range(N):
                        t0, t1 = n * NC_CHUNK, (n + 1) * NC_CHUNK
                        ps_k = psp.tile([128, NC_CHUNK], F32, name=f"psk{b}_{o}_{n}", tag="ps")
                        for d in range(ND):
                            nc.tensor.matmul(
                                ps_k[:], wkv_sb[:, d * 128:(d + 1) * 128],
                                X[d][:, t0:t1], start=(d == 0), stop=(d == ND - 1))
                        ps_v = psp.tile([128, NC_CHUNK], F32, name=f"psv{b}_{o}_{n}", tag="ps")
                        for d in range(ND):
                            nc.tensor.matmul(
                                ps_v[:], wkv_sb[:, D + d * 128:D + (d + 1) * 128],
                                X[d][:, t0:t1], start=(d == 0), stop=(d == ND - 1))
                        # kf = exp(min(k+bk,0)) + relu(k+bk)
                        y1 = bscr.tile([128, NC_CHUNK], BF16, name=f"y1_{b}_{o}_{n}", tag="bscr")
                        nc.scalar.activation(y1[:], ps_k[:], AF.Relu,
                                             bias=scol("nbk", o), scale=-1.0)
                        t2 = bscr.tile([128, NC_CHUNK], BF16, name=f"t2_{b}_{o}_{n}", tag="bscr")
                        nc.scalar.activation(t2[:], ps_k[:], AF.Relu, bias=scol("bk", o))
                        nc.scalar.activation(y1[:], y1[:], AF.Exp, scale=-1.0)
                        vsb = scr.tile([128, NC_CHUNK], BF16, name=f"vs_{b}_{o}_{n}", tag="vsb")
                        nc.vector.tensor_copy(vsb[:], ps_v[:])
                        # fp32 out: accum_out precision follows the out dtype
                        kf = fscr.tile([128, NC_CHUNK], F32, name=f"kf_{b}_{o}_{n}", tag="fscr")
                        nc.vector.scalar_tensor_tensor(
                            kf[:], y1[:], 0.0, t2[:], ALU.add, ALU.add,
                            accum_out=kvs_loc[:, o * 8 + 4 + n:o * 8 + 5 + n])
                        nc.vector.scalar_tensor_tensor(
                            kf[:], kf[:], 1.0, vsb[:], ALU.mult, ALU.mult,
                            accum_out=kvs_loc[:, o * 8 + n:o * 8 + 1 + n])

                # ---------------- AllReduce of KV/sumK --------------------
                cc_in = dramp.tile([128, 2 * NO * N], F32, name=f"cci{b}", tag="cci")
                cc_out = dramp.tile([128, 2 * NO * N], F32, name=f"cco{b}", tag="cco")
                nc.sync.dma_start(cc_in[:], kvs_loc[:])
                nc.gpsimd.collective_compute(
                    "AllReduce", mybir.AluOpType.add,
                    replica_groups=[list(range(N_CORES))],
                    ins=[cc_in.opt()], outs=[cc_out.opt()])
                kvs = kvsp.tile([128, 2 * NO * N], F32, name=f"kvg{b}", tag="kvg")
                nc.sync.dma_start(kvs[:], cc_out[:])
                # fold V-bias into KV: kv += bv * sumk  (exact)
                for o in range(NO):
                    nc.vector.scalar_tensor_tensor(
                        kvs[:, o * 8:o * 8 + 4], kvs[:, o * 8 + 4:o * 8 + 8],
                        scol("bv", o), kvs[:, o * 8:o * 8 + 4], ALU.mult, ALU.add)
                if debug and b == 0:
                    nc.sync.dma_start(dbg_aps["d_kvs"][:], kvs[:])

                # ------------- Phase B, split in n-halves -----------------
                for nh in range(2):
                    ns = (2 * nh, 2 * nh + 1)
                    qf = {}
                    # B1: Q projections + feature map (independent of the CC)
                    for o in range(NO):
                        wq_sb = wp.tile([128, D], BF16, name=f"wq{b}_{o}_{nh}", tag="wq")
                        nc.scalar.dma_start(wq_sb[:], wq_ap[b, o])
                        for n in ns:
                            t0, t1 = n * NC_CHUNK, (n + 1) * NC_CHUNK
                            ps_q = psp.tile([128, NC_CHUNK], F32, name=f"psq{b}_{o}_{n}", tag="ps")
                            for d in range(ND):
                                nc.tensor.matmul(
                                    ps_q[:], wq_sb[:, d * 128:(d + 1) * 128],
                                    X[d][:, t0:t1], start=(d == 0), stop=(d == ND - 1))
                            y1q = bscr.tile([128, NC_CHUNK], BF16, name=f"yq_{b}_{o}_{n}", tag="bscr")
                            nc.scalar.activation(y1q[:], ps_q[:], AF.Relu,
                                                 bias=scol("nbq", o), scale=-1.0)
                            t2q = bscr.tile([128, NC_CHUNK], BF16, name=f"qt2_{b}_{o}_{n}", tag="bscr")
                            nc.scalar.activation(t2q[:], ps_q[:], AF.Relu, bias=scol("bq", o))
                            nc.scalar.activation(y1q[:], y1q[:], AF.Exp, scale=-1.0)
                            qt = qfp.tile([128, NC_CHUNK], BF16, name=f"qf{b}_{o}_{n}", tag=f"qf{o}")
                            nc.vector.tensor_tensor(qt[:], y1q[:], t2q[:], ALU.add)
                            qf[(o, n)] = qt

                    # B2: att = qf * kv / (qf*sumk + eps), in place on qf
                    for n in ns:
                        for o in range(NO):
                            qt = qf[(o, n)]
                            den = fscr.tile([128, NC_CHUNK], F32, name=f"dn_{b}_{o}_{n}", tag="fscr")
                            # on GpSimd: runs parallel to the DVE recip/att chain
                            nc.gpsimd.tensor_scalar(
                                den[:], qt[:], kvs[:, o * 8 + 4 + n:o * 8 + 5 + n],
                                ATT_EPS, ALU.mult, ALU.add)
                            nc.vector.reciprocal_approx_fast(den[:], den[:])
                            # recb = kv / den via the ACT per-partition scale
                            recb = bscr.tile([128, NC_CHUNK], BF16, name=f"rb_{b}_{o}_{n}", tag="bscr")
                            nc.scalar.activation(recb[:], den[:], AF.Copy,
                                                 scale=kvs[:, o * 8 + n:o * 8 + 1 + n])
                            nc.vector.tensor_tensor(qt[:], qt[:], recb[:], ALU.mult)
                            if debug and b == 0 and o == 0:
                                t0, t1 = n * NC_CHUNK, (n + 1) * NC_CHUNK
                                nc.sync.dma_start(dbg_aps["d_att0"][:, t0:t1], qt[:])

                    # B3: Wo -> gelu -> z = y + x
                    for o2 in range(ND):
                        wo_sb = wop.tile([128, HK], BF16, name=f"wo{b}_{o2}_{nh}", tag="wo")
                        nc.sync.dma_start(wo_sb[:], wo_ap[b, o2])
                        for n in ns:
                            t0, t1 = n * NC_CHUNK, (n + 1) * NC_CHUNK
                            ps_y = psp.tile([128, NC_CHUNK], F32, name=f"psy{b}_{o2}_{n}", tag="ps")
                            for o in range(NO):
                                nc.tensor.matmul(
                                    ps_y[:], wo_sb[:, o * 128:(o + 1) * 128],
                                    qf[(o, n)][:], start=(o == 0), stop=(o == NO - 1))
                            yt = actp.tile([128, NC_CHUNK], BF16, name=f"y_{b}_{o2}_{n}", tag="actout")
                            nc.scalar.activation(yt[:], ps_y[:], AF.Gelu_apprx_tanh,
                                                 bias=scol("bo", o2))
                            nc.vector.tensor_tensor(Z[o2][:, t0:t1], yt[:], Z[o2][:, t0:t1],
                                                    ALU.add)

                if debug and b == 0:
                    nc.sync.dma_start(dbg_aps["d_z1"][:], Z[0][:].bitcast(F32))

                # ---------------- LN1: x = LN(z) --------------------------
                ln("ln1", b, scol)

                if debug and b == 0:
                    nc.sync.dma_start(dbg_aps["d_ln1"][:], X[0][:])

                # ------- FFN: z2 = gelu(gelu(x@W1+b1)@W2+b2) + x ----------
                for c in range(N):
                    t0, t1 = c * NC_CHUNK, (c + 1) * NC_CHUNK
                    hs = []
                    for f in range(NF):
                        w1t = w1p.tile([128, D], BF16, name=f"w1_{b}_{c}_{f}", tag="w1")
                        (nc.sync if f % 2 == 0 else nc.scalar).dma_start(
                            w1t[:], w1_ap[b, f])
                        ps_h = psp.tile([128, NC_CHUNK], F32, name=f"psh{b}_{c}_{f}", tag="ps")
                        for d in range(ND):
                            nc.tensor.matmul(
                                ps_h[:], w1t[:, d * 128:(d + 1) * 128],
                                X[d][:, t0:t1], start=(d == 0), stop=(d == ND - 1))
                        ht = hp.tile([128, NC_CHUNK], BF16, name=f"h_{b}_{c}_{f}", tag="h")
                        nc.scalar.activation(ht[:], ps_h[:], AF.Gelu_apprx_tanh,
                                             bias=scol("b1", f))
                        if debug and b == 0 and f == 0 and c == 0:
                            nc.sync.dma_start(dbg_aps["d_h0"][:], ht[:])
                        hs.append(ht)
                    for o4 in range(ND):
                        ps2 = psp.tile([128, NC_CHUNK], F32, name=f"ps2{b}_{c}_{o4}", tag="ps")
                        for quart in range(4):
                            w2t = w2p.tile([128, F // 4], BF16,
                                          name=f"w2_{b}_{c}_{o4}_{quart}", tag="w2")
                            nc.gpsimd.dma_start(w2t[:], w2_ap[b, o4, quart])
                            for j in range(NF // 4):
                                f = quart * (NF // 4) + j
                                nc.tensor.matmul(
                                    ps2[:], w2t[:, j * 128:(j + 1) * 128], hs[f][:],
                                    start=(f == 0), stop=(f == NF - 1))
                        gt = actp.tile([128, NC_CHUNK], BF16, name=f"g_{b}_{c}_{o4}", tag="actout")
                        nc.scalar.activation(gt[:], ps2[:], AF.Gelu_apprx_tanh,
                                             bias=scol("b2", o4))
                        nc.vector.tensor_tensor(Z[o4][:, t0:t1], gt[:], Z[o4][:, t0:t1],
                                                ALU.add)

                if debug and b == 0:
                    nc.sync.dma_start(dbg_aps["d_ffnz"][:], Z[0][:].bitcast(F32))

                # ---------------- LN2: x = LN(z2) -------------------------
                ln("ln2", b, scol, final=(b == n_blocks - 1))

    nc.compile()
    return nc


def _prep_inputs(inputs, n_blocks):
    """Host-side: shard x over sequence, pre-transpose to feature-major
    bf16, pack weights as contiguous lhsT tiles, pack biases/scales."""
    import ml_dtypes

    bf16 = ml_dtypes.bfloat16
    x = np.asarray(inputs["x"], dtype=np.float32)
    Wq = np.asarray(inputs["Wq"], dtype=np.float32)
    Wk = np.asarray(inputs["Wk"], dtype=np.float32)
    Wv = np.asarray(inputs["Wv"], dtype=np.float32)
    Wo = np.asarray(inputs["Wo"], dtype=np.float32)
    W1 = np.asarray(inputs["W1"], dtype=np.float32)
    W2 = np.asarray(inputs["W2"], dtype=np.float32)

    def qkv_pack(arr):  # [B,H,D,K] -> [B,D,HK] -> [nb, o, p(d), dd, m(hk)]
        a2 = arr.transpose(0, 2, 1, 3).reshape(B, D, HK)[:n_blocks]
        a5 = a2.reshape(n_blocks, ND, 128, NO, 128)     # [b, dd, p(d), o, m(hk)]
        # lhsT tile[p(d in dd), dd*128+m(hk)] = W[b, d=dd*128+p, hk=o*128+m]
        return a5.transpose(0, 3, 2, 1, 4).astype(bf16)  # [b, o, p, dd, m]

    wq = np.ascontiguousarray(qkv_pack(Wq))
    wkv = np.ascontiguousarray(
        np.concatenate([qkv_pack(Wk).reshape(n_blocks, NO, 128, D),
                        qkv_pack(Wv).reshape(n_blocks, NO, 128, D)], axis=3))
    # Wo [B, HK, D]: tile[o2][p(hk in o), o*128+m(d in o2)] = Wo[b, o*128+p, o2*128+m]
    wo = np.ascontiguousarray(
        Wo[:n_blocks].reshape(n_blocks, NO, 128, ND, 128)
        .transpose(0, 3, 2, 1, 4).astype(bf16))          # [b, o2, p, o, m]
    # W1 [B, D, F]: tile[f][p(d in dd), dd*128+m(f)] = W1[b, d=dd*128+p, f=f*128+m]
    w1 = np.ascontiguousarray(
        W1[:n_blocks].reshape(n_blocks, ND, 128, NF, 128)
        .transpose(0, 3, 2, 1, 4).astype(bf16))          # [b, f, p, dd, m]
    # W2 [B, F, D]: tile[o4][p(f in ft), ft*128+m(d in o4)] = W2[b, ft*128+p, o4*128+m]
    w2 = np.ascontiguousarray(
        W2[:n_blocks].reshape(n_blocks, NF, 128, ND, 128)
        .transpose(0, 3, 2, 1, 4)                        # [b, o4, p, ft, m]
        .reshape(n_blocks, ND, 128, 4, (NF // 4) * 128)  # split f-quarters
        .transpose(0, 1, 3, 2, 4).astype(bf16))          # [b, o4, quart, p, cols]

    scal = np.zeros((n_blocks, 128, NSCAL), np.float32)

    def put(name, arr2d):  # arr2d [n_blocks, width] -> 128-chunk columns
        w = arr2d.shape[1]
        ncol = w // 128
        scal[:, :, _COLS[name]:_COLS[name] + ncol] = \
            arr2d.reshape(n_blocks, ncol, 128).transpose(0, 2, 1)

    bq2 = np.asarray(inputs["bq"], np.float32).reshape(B, HK)[:n_blocks]
    bk2 = np.asarray(inputs["bk"], np.float32).reshape(B, HK)[:n_blocks]
    put("bq", bq2)
    put("bk", bk2)
    put("nbq", -bq2)
    put("nbk", -bk2)
    put("bv", np.asarray(inputs["bv"], np.float32).reshape(B, HK)[:n_blocks])
    put("bo", np.asarray(inputs["bo"], np.float32)[:n_blocks])
    put("b1", np.asarray(inputs["b1"], np.float32)[:n_blocks])
    put("b2", np.asarray(inputs["b2"], np.float32)[:n_blocks])
    put("ln1_s", np.asarray(inputs["ln1_s"], np.float32)[:n_blocks])
    put("ln1_b", np.asarray(inputs["ln1_b"], np.float32)[:n_blocks])
    put("ln2_s", np.asarray(inputs["ln2_s"], np.float32)[:n_blocks])
    put("ln2_b", np.asarray(inputs["ln2_b"], np.float32)[:n_blocks])

    ln_bias = bool(
        np.any(np.asarray(inputs["ln1_b"])[:n_blocks]) or
        np.any(np.asarray(inputs["ln2_b"])[:n_blocks]))
    ln_affine = ln_bias or bool(
        np.any(np.asarray(inputs["ln1_s"])[:n_blocks] != 1.0) or
        np.any(np.asarray(inputs["ln2_s"])[:n_blocks] != 1.0))

    in_maps = []
    for core in range(N_CORES):
        s0 = core * SEQ_SH
        xc = np.ascontiguousarray(
            x[:, s0:s0 + SEQ_SH, :].transpose(2, 0, 1).reshape(D, TPC).astype(bf16))
        in_maps.append({
            "x_fm": xc, "wkv": wkv, "wq": wq, "wo": wo,
            "w1": w1, "w2": w2, "scal": scal,
        })
    return in_maps, ln_affine, ln_bias


def run(inputs, n_blocks=B, trace=False, debug=False):
    from concourse.bass_utils import run_bass_kernel_spmd

    in_maps, ln_affine, ln_bias = _prep_inputs(inputs, n_blocks)
    key = (n_blocks, ln_affine, ln_bias, debug)
    if key not in _BUILD_CACHE:
        _BUILD_CACHE[key] = _build(n_blocks, ln_affine=ln_affine,
                                   ln_bias=ln_bias, debug=debug)
    nc = _BUILD_CACHE[key]
    res = run_bass_kernel_spmd(nc, in_maps, list(range(N_CORES)), trace=trace)
    # gather: per-core [D, TPC] feature-major -> [N, S, D]
    out = np.empty((N, S, D), np.float32)
    for core in range(N_CORES):
        s0 = core * SEQ_SH
        oc = np.asarray(res.results[core]["out_fm"]).astype(np.float32)  # [D, TPC]
        out[:, s0:s0 + SEQ_SH, :] = \
            oc.reshape(D, N, SEQ_SH).transpose(1, 2, 0)
    return out, res


def kernel(**inputs):
    out, _ = run(inputs, n_blocks=B, trace=False)
    return out
